# revision 1
# baseline (speedup 1.0000x reference)
"""Trainium2 Bass kernel for nn_DecoderStack — v3.

Key structural ideas vs v2 baseline:
  * sub_norm algebra: the nested sub_norms collapse exactly to ONE final
    sub_norm(y + mha1 + mha2 + ffn + b_out) because sub_norm subtracts only
    per-row constants (mean+std) and both are shift-equivariant/invariant.
    So the three branches are INDEPENDENT and the 77us exposed tail chain of
    v2 disappears.
  * Attention: per-core 8 heads over full T (batch b=c//2, head-half h=c%2).
    Instead of ReduceScatter after Wo, exchange partial activations
    (AllToAll over the core pair) BEFORE Wo; each core then computes Wo only
    for its own TH=512 rows with the full 1024 contract -> half the Wo work
    and the collective rides under compute.
  * FFN: own rows x full FF on every core (full W_in/W_out streamed) -> no
    collective at all for the FFN branch.
  * Scores matmuls (64-wide contract) are issued in head PAIRS to distinct
    PE row groups (rows 0:64 / 64:128) -> they run concurrently.
  * exps / Q / K / V tiles in fp8e4m3 (plenty of error headroom; halves SBUF
    and lets the softmax pipeline run deep).
  * PE warmed up with junk matmuls at t=0 (HAM clock gate) while DMAs land.
  * Scalar engine exp chain (~1.5us per [128,1024] exp) is the serial floor
    of the head phases; FFN h1 / QKV2 / Wo1 work is hand-interleaved into the
    head loops to keep the PE busy behind it.
"""

import sys

for _p in ("/opt/trn_rl_repo", "/root/.axon_site"):
    if _p not in sys.path:
        sys.path.insert(0, _p)

import contextlib

import numpy as np

import concourse.bass as bass
import concourse.bacc as bacc
import concourse.tile as tile
from concourse import mybir
from concourse.bass_utils import run_bass_kernel_spmd

B, T, D, H, DK, DV, FF = 4, 1024, 1024, 16, 64, 64, 4096
P = 128
TH = T // 2           # rows owned per core
NT = T // P           # 8 t/s tiles over full T
ND = D // P           # 8 d chunks
NF = FF // P          # 32 ff chunks
NTO = TH // P         # 4 own-row tiles
FP32 = mybir.dt.float32
BF16 = mybir.dt.bfloat16
FP8 = mybir.dt.float8e4
NPBF16 = mybir.dt.np(BF16)
WV_SCALE = 48.0       # fp8: wv*48 keeps |V| < 240 (TRN e4m3 max, inf beyond)
WVP_BOOST = 8.0       # extra boost on wvp (= wva/denom) to avoid denormals
RS_SCALE = 8.0        # m-branch RS payload in fp8 at 8x scale


def build_program(n_cores: int = 8, compile: bool = True):
    nc = bacc.Bacc("TRN2", target_bir_lowering=False, debug=False,
                   num_devices=n_cores)
    groups = [[2 * g, 2 * g + 1] for g in range(n_cores // 2)]

    def dram_in(name, shape, dt=BF16):
        return nc.dram_tensor(name, shape, dt, kind="ExternalInput")

    yT = dram_in("yT", [P, ND, T])
    xT = dram_in("xT", [P, ND, T])
    yTo = dram_in("yTo", [P, ND, TH])
    ynb = dram_in("ynb", [TH, D], FP32)        # y own rows + b_out
    wq1 = dram_in("wq1", [P, ND, 512])
    wk1 = dram_in("wk1", [P, ND, 512])
    wv1 = dram_in("wv1", [P, ND, 512])
    wo1 = dram_in("wo1", [P, 4, D])
    wq2 = dram_in("wq2", [P, ND, 512])
    wk2 = dram_in("wk2", [P, ND, 512])
    wv2 = dram_in("wv2", [P, ND, 512])
    wo2 = dram_in("wo2", [P, 4, D])
    wi = dram_in("wi", [NF, P, ND, P])
    wot = dram_in("wot", [NF, P, D])
    bi = dram_in("bi", [P, NF], FP32)
    out = nc.dram_tensor("out", [TH, D], FP32, kind="ExternalOutput")

    with tile.TileContext(nc) as tc:
        with contextlib.ExitStack() as ctx:
            p1 = ctx.enter_context(tc.tile_pool(name="p1", bufs=1))
            expp = ctx.enter_context(tc.tile_pool(name="expp", bufs=20))
            small = ctx.enter_context(tc.tile_pool(name="small", bufs=2))
            psum = ctx.enter_context(tc.tile_pool(name="psum", bufs=2, space="PSUM"))
            dram = ctx.enter_context(tc.tile_pool(name="dram", bufs=1, space="DRAM"))

            rs1_in = dram.tile([T, D], FP8, tag="rs1i", name="rs1_in")
            rs1_out = dram.tile([TH, D], FP8, tag="rs1o", name="rs1_out")
            rs2_in = dram.tile([T, D], FP8, tag="rs2i", name="rs2_in")
            rs2_out = dram.tile([TH, D], FP8, tag="rs2o", name="rs2_out")

            # ---------------- warmup (HAM) + persistent loads
            junk = p1.tile([P, 512], BF16, tag="junk")
            nc.gpsimd.memset(junk[:], 0.25)
            jp = [psum.tile([P, 512], FP32, tag="st", name=f"jp{i}")
                  for i in range(2)]
            for i in range(44):
                nc.tensor.matmul(jp[i % 2][:], lhsT=junk[:, 0:P], rhs=junk[:],
                                 start=True, stop=True, skip_group_check=True)

            yT_sb = p1.tile([P, ND, T], BF16, tag="yT")
            for dc in range(ND):
                nc.sync.dma_start(yT_sb[:, dc, :], yT[:, dc, :])
            wq_sb = p1.tile([P, ND, 512], BF16, tag="wq", name="wq1_sb")
            wk_sb = p1.tile([P, ND, 512], BF16, tag="wk", name="wk1_sb")
            nc.sync.dma_start(wq_sb[:], wq1[:])
            nc.sync.dma_start(wk_sb[:], wk1[:])
            S = p1.tile([P, NTO, D], FP32, tag="S")
            for t_ in range(NTO):
                nc.sync.dma_start(S[:, t_, :], ynb[t_ * P:(t_ + 1) * P, :])
            wv_sb = p1.tile([P, ND, 512], BF16, tag="wv", name="wv1_sb")
            nc.sync.dma_start(wv_sb[:], wv1[:])
            xT_sb = p1.tile([P, ND, T], BF16, tag="xT")
            for dc in range(ND):
                nc.sync.dma_start(xT_sb[:, dc, :], xT[:, dc, :])
            yTo_sb = p1.tile([P, ND, TH], BF16, tag="yTo")
            nc.sync.dma_start(yTo_sb[:], yTo[:])
            bi_sb = p1.tile([P, NF], FP32, tag="bi")
            nc.sync.dma_start(bi_sb[:], bi[:])

            # ---------------- building blocks
            def qk_pair(wsb, dst, p, rhs_sb):
                """Project one 128-col block (head pair p) of Q or K.
                Uses a 2-bank "sc" psum tile; only safe OUTSIDE head loops."""
                ps = psum.tile([P, T], FP32, tag="sc", name=f"qk_{dst.name}_{p}")
                for dc in range(ND):
                    for th in range(2):
                        nc.tensor.matmul(
                            ps[:, th * TH:(th + 1) * TH],
                            lhsT=wsb[:, dc, p * P:(p + 1) * P],
                            rhs=rhs_sb[:, dc, th * TH:(th + 1) * TH],
                            start=(dc == 0), stop=(dc == ND - 1),
                            skip_group_check=True)
                nc.vector.tensor_copy(out=dst[:, p, :], in_=ps[:])

            def qk_half(wsb, dst, p, th, rhs_sb):
                """Stuffable half projection using a 1-bank "st" tile."""
                ps = psum.tile([P, TH], FP32, tag="st",
                               name=f"qkh_{dst.name}_{p}_{th}")
                for dc in range(ND):
                    nc.tensor.matmul(
                        ps[:], lhsT=wsb[:, dc, p * P:(p + 1) * P],
                        rhs=rhs_sb[:, dc, th * TH:(th + 1) * TH],
                        start=(dc == 0), stop=(dc == ND - 1))
                nc.vector.tensor_copy(out=dst[:, p, th * TH:(th + 1) * TH],
                                      in_=ps[:])

            def v_unit(wva, st, kv_sb):
                pv = psum.tile([P, 512], FP32, tag="st", name=f"v_{wva.name}_{st}")
                for dc in range(ND):
                    nc.tensor.matmul(
                        pv[:], lhsT=kv_sb[:, dc, st * P:(st + 1) * P],
                        rhs=wv_sb[:, dc, :],
                        start=(dc == 0), stop=(dc == ND - 1))
                nc.vector.tensor_copy(out=wva[:, st, :], in_=pv[:])

            def h1_unit(h1_sb, fc):
                wi_c = p1.tile([P, ND, P], BF16, tag="wic", bufs=3,
                               name=f"wic_{fc}")
                nc.sync.dma_start(wi_c[:], wi[fc])
                ph = psum.tile([P, TH], FP32, tag="st", name=f"h1_{fc}")
                for dc in range(ND):
                    nc.tensor.matmul(
                        ph[:], lhsT=wi_c[:, dc, :], rhs=yTo_sb[:, dc, :],
                        start=(dc == 0), stop=(dc == ND - 1))
                nc.vector.tensor_scalar(
                    out=h1_sb[:, fc, :], in0=ph[:],
                    scalar1=bi_sb[:, fc:fc + 1], scalar2=0.0,
                    op0=mybir.AluOpType.add, op1=mybir.AluOpType.max)

            def wo_unit(pt, wof, tt8, dh, dst_ap):
                """m-partial rows tt8 (full T), d-half dh -> fp8 -> DRAM rs buf."""
                ps = psum.tile([P, TH], FP32, tag="st",
                               name=f"wo_{wof.name}_{tt8}_{dh}")
                for i in range(4):
                    nc.tensor.matmul(
                        ps[:], lhsT=pt[:, i, tt8 * P:(tt8 + 1) * P],
                        rhs=wof[:, i, dh * TH:(dh + 1) * TH],
                        start=(i == 0), stop=(i == 3))
                stg = p1.tile([P, TH], FP8, tag="mstg", bufs=3,
                              name=f"mstg_{wof.name}_{tt8}_{dh}")
                nc.vector.tensor_copy(out=stg[:], in_=ps[:])
                nc.sync.dma_start(dst_ap, stg[:])

            def m_add(rs_out_ap, tt, tag):
                """S[tt] += rs_out / RS_SCALE (one own-row tile).
                The load rides the gpsimd DMA queue so a pending collective
                does not head-of-line-block the sync-engine DMA stream."""
                mld = p1.tile([P, D], FP8, tag="mld", bufs=4,
                              name=f"mld_{tag}_{tt}")
                nc.gpsimd.dma_start(mld[:], rs_out_ap)
                nc.vector.scalar_tensor_tensor(
                    out=S[:, tt, :], in0=mld[:], scalar=1.0 / RS_SCALE,
                    in1=S[:, tt, :],
                    op0=mybir.AluOpType.mult, op1=mybir.AluOpType.add)

            # stuffing queue machinery
            stuff_q = []

            def maybe_stuff(n=1):
                for _ in range(n):
                    if stuff_q:
                        stuff_q.pop(0)()

            def heads(m, wqt, wkt, wva, pt):
                """4 head-pairs; scores row-group paired; exp fp8; partial."""
                prev = None

                def partial_block(p, exA, exB, den):
                    rden = small.tile([P, 2, NT], FP32, tag="rden",
                                      name=f"rden{m}_{p}")
                    nc.vector.reciprocal(out=rden[:], in_=den[:])
                    wvp = small.tile([P, 2, NT, DV], FP8, tag="wvp",
                                     name=f"wvp{m}_{p}")
                    for j in range(2):
                        for st in range(NT):
                            # wvp = (wva / denom) * WVP_BOOST: keeps fp8
                            # values out of the denormal zone
                            nc.vector.tensor_scalar(
                                out=wvp[:, j, st, :],
                                in0=wva[:, st, (2 * p + j) * DV:(2 * p + j + 1) * DV],
                                scalar1=rden[:, j, st:st + 1],
                                scalar2=WVP_BOOST,
                                op0=mybir.AluOpType.mult,
                                op1=mybir.AluOpType.mult)
                    pps = [psum.tile([P, TH], FP32, tag="pp", name=f"pp{m}_{p}_{j}")
                           for j in range(2)]
                    for j, ex in ((0, exA), (1, exB)):
                        for st in range(NT):
                            nc.tensor.matmul(
                                pps[j][0:64, :], lhsT=wvp[:, j, st, :],
                                rhs=ex[st][:, 0:TH],
                                start=(st == 0), stop=(st == NT - 1),
                                skip_group_check=True)
                            nc.tensor.matmul(
                                pps[j][64:128, :], lhsT=wvp[:, j, st, :],
                                rhs=ex[st][:, TH:T],
                                start=(st == 0), stop=(st == NT - 1),
                                tile_position=(0, 64), skip_group_check=True)
                    for j in range(2):
                        lo, hi = 64 * j, 64 * j + 64
                        nc.vector.tensor_copy(out=pt[lo:hi, p, 0:TH],
                                              in_=pps[j][0:64, :])
                        nc.vector.tensor_copy(out=pt[lo:hi, p, TH:T],
                                              in_=pps[j][64:128, :])

                for p in range(4):
                    den = small.tile([P, 2, NT], FP32, tag="den",
                                     name=f"den{m}_{p}")
                    exA, exB = [], []
                    for st in range(NT):
                        psA = psum.tile([P, T], FP32, tag="sc",
                                        name=f"scA{m}_{p}_{st}")
                        psB = psum.tile([P, T], FP32, tag="sc",
                                        name=f"scB{m}_{p}_{st}")
                        for th in range(2):
                            tsl = slice(th * TH, (th + 1) * TH)
                            nc.tensor.matmul(
                                psA[:, tsl],
                                lhsT=wkt[0:64, p, st * P:(st + 1) * P],
                                rhs=wqt[0:64, p, tsl],
                                start=True, stop=True, skip_group_check=True)
                            nc.tensor.matmul(
                                psB[:, tsl],
                                lhsT=wkt[64:128, p, st * P:(st + 1) * P],
                                rhs=wqt[64:128, p, tsl],
                                start=True, stop=True, skip_group_check=True)
                        eA = expp.tile([P, T], FP8, tag="exp",
                                       name=f"exA{m}_{p}_{st}")
                        nc.scalar.activation(
                            out=eA[:], in_=psA[:],
                            func=mybir.ActivationFunctionType.Exp,
                            accum_out=den[:, 0, st:st + 1])
                        eB = expp.tile([P, T], FP8, tag="exp",
                                       name=f"exB{m}_{p}_{st}")
                        nc.scalar.activation(
                            out=eB[:], in_=psB[:],
                            func=mybir.ActivationFunctionType.Exp,
                            accum_out=den[:, 1, st:st + 1])
                        exA.append(eA)
                        exB.append(eB)
                        if st < 7:
                            maybe_stuff(1)
                    if prev is not None:
                        partial_block(*prev)
                        maybe_stuff(3)
                    prev = (p, exA, exB, den)
                partial_block(*prev)
                maybe_stuff(3)

            # ---------------- QKV1 projections (PE dense from the start)
            wqt1 = p1.tile([P, 4, T], FP8, tag="wqt", bufs=2, name="wqt1")
            wkt1 = p1.tile([P, 4, T], FP8, tag="wkt", bufs=2, name="wkt1")
            for p in range(4):
                qk_pair(wq_sb, wqt1, p, yT_sb)
                qk_pair(wk_sb, wkt1, p, yT_sb)

            wva1 = p1.tile([P, NT, 512], FP8, tag="wva", bufs=2, name="wva1")
            wva2 = p1.tile([P, NT, 512], FP8, tag="wva", bufs=2, name="wva2")
            h1_sb = p1.tile([P, NF, TH], BF16, tag="h1")

            wq2_sb = p1.tile([P, ND, 512], BF16, tag="wq", name="wq2_sb")
            wk2_sb = p1.tile([P, ND, 512], BF16, tag="wk", name="wk2_sb")
            wv2_sb = p1.tile([P, ND, 512], BF16, tag="wv", name="wv2_sb")
            wqt2 = p1.tile([P, 4, T], FP8, tag="wqt", bufs=2, name="wqt2")
            wkt2 = p1.tile([P, 4, T], FP8, tag="wkt", bufs=2, name="wkt2")

            def load_w2():
                nc.sync.dma_start(wq2_sb[:], wq2[:])
                nc.sync.dma_start(wk2_sb[:], wk2[:])

            def load_wv2():
                nc.sync.dma_start(wv2_sb[:], wv2[:])

            def qk2_units(p):
                return [lambda th=th: qk_half(wq2_sb, wqt2, p, th, yT_sb)
                        for th in range(2)] + \
                       [lambda th=th: qk_half(wk2_sb, wkt2, p, th, xT_sb)
                        for th in range(2)]

            # heads1 stuffing: v1 / v2 / qk2 p0-p3 / h1 (tail spills to heads2)
            for st in range(NT):
                stuff_q.append(lambda st=st: v_unit(wva1, st, yT_sb))
            stuff_q.append(load_w2)
            stuff_q.append(load_wv2)
            stuff_q.extend(qk2_units(0))
            for st in range(NT):
                stuff_q.append(lambda st=st: v_unit(wva2, st, xT_sb))
            stuff_q.extend(qk2_units(1))
            for fc in range(8):
                stuff_q.append(lambda fc=fc: h1_unit(h1_sb, fc))
            stuff_q.extend(qk2_units(2))
            stuff_q.extend(qk2_units(3))
            for fc in range(8, NF):
                stuff_q.append(lambda fc=fc: h1_unit(h1_sb, fc))

            # ---------------- heads1 (scalar-bound; stuffed)
            pt1 = p1.tile([P, 4, T], BF16, tag="pt", name="pt1")
            heads(1, wqt1, wkt1, wva1, pt1)

            # ---------------- m1 = pt1 @ Wo1 (full T) -> RS1, stuffed into
            # heads2; the cc fires mid-heads2 from a stuffed closure so the
            # reduce is hidden under the second head phase.
            wo1f = p1.tile([P, 4, D], BF16, tag="wof", bufs=2, name="wo1f")
            nc.sync.dma_start(wo1f[:], wo1[:])
            wo2f = p1.tile([P, 4, D], BF16, tag="wof", bufs=2, name="wo2f")
            nc.sync.dma_start(wo2f[:], wo2[:])

            for tt8 in range(NT):
                for dh in range(2):
                    stuff_q.append(lambda tt8=tt8, dh=dh: wo_unit(
                        pt1, wo1f, tt8, dh,
                        rs1_in[tt8 * P:(tt8 + 1) * P, dh * TH:(dh + 1) * TH]))

            def rs1_cc():
                nc.gpsimd.collective_compute(
                    "ReduceScatter", mybir.AluOpType.add, replica_groups=groups,
                    ins=[rs1_in.opt()], outs=[rs1_out.opt()])

            stuff_q.append(rs1_cc)

            # ---------------- heads2
            pt2 = p1.tile([P, 4, T], BF16, tag="pt", name="pt2")
            heads(2, wqt2, wkt2, wva2, pt2)
            while stuff_q:
                maybe_stuff(1)

            # ---------------- m2 -> RS2 ; m1 adds
            for tt8 in range(NT):
                for dh in range(2):
                    wo_unit(pt2, wo2f, tt8, dh,
                            rs2_in[tt8 * P:(tt8 + 1) * P,
                                   dh * TH:(dh + 1) * TH])
            nc.gpsimd.collective_compute(
                "ReduceScatter", mybir.AluOpType.add, replica_groups=groups,
                ins=[rs2_in.opt()], outs=[rs2_out.opt()])
            for tt in range(NTO):
                m_add(rs1_out[tt * P:(tt + 1) * P, :], tt, "m1")

            def final_chain(tt):
                stats = small.tile([P, 2, 6], FP32, tag="stats",
                                   name=f"stats_{tt}")
                for i in range(2):
                    nc.vector.bn_stats(out=stats[:, i, :],
                                       in_=S[:, tt, i * TH:(i + 1) * TH])
                mv = small.tile([P, 2], FP32, tag="mv", name=f"mv_{tt}")
                nc.vector.bn_aggr(out=mv[:], in_=stats[:])
                std = small.tile([P, 1], FP32, tag="std", name=f"std_{tt}")
                nc.scalar.activation(
                    out=std[:], in_=mv[:, 1:2],
                    func=mybir.ActivationFunctionType.Sqrt,
                    scale=float(D) / float(D - 1))
                msum = small.tile([P, 1], FP32, tag="msum", name=f"msum_{tt}")
                nc.vector.tensor_add(out=msum[:], in0=mv[:, 0:1], in1=std[:])
                nc.vector.tensor_scalar_sub(out=S[:, tt, :], in0=S[:, tt, :],
                                            scalar1=msum[:])
                nc.sync.dma_start(out[tt * P:(tt + 1) * P, :], S[:, tt, :])

            # ffp sweep over (tt pair) halves; m2 add + final chain at end
            def ffp_sweep(tts, _unused):
                acc = [psum.tile([P, T], FP32, tag="sc", name=f"ffa_{tts[0]}_{k}")
                       for k in range(2)]
                for fc in range(NF):
                    wot_c = p1.tile([P, D], BF16, tag="wotc", bufs=4,
                                    name=f"wotc_{tts[0]}_{fc}")
                    nc.sync.dma_start(wot_c[:], wot[fc])
                    for k, tt in enumerate(tts):
                        for dh in range(2):
                            nc.tensor.matmul(
                                acc[k][:, dh * TH:(dh + 1) * TH],
                                lhsT=h1_sb[:, fc, tt * P:(tt + 1) * P],
                                rhs=wot_c[:, dh * TH:(dh + 1) * TH],
                                start=(fc == 0), stop=(fc == NF - 1),
                                skip_group_check=True)
                for k, tt in enumerate(tts):
                    for dh in range(2):
                        nc.vector.tensor_add(
                            out=S[:, tt, dh * TH:(dh + 1) * TH],
                            in0=acc[k][:, dh * TH:(dh + 1) * TH],
                            in1=S[:, tt, dh * TH:(dh + 1) * TH])
                for tt in tts:
                    m_add(rs2_out[tt * P:(tt + 1) * P, :], tt, "m2")
                    final_chain(tt)

            ffp_sweep((0, 1), None)
            ffp_sweep((2, 3), None)

    if compile:
        nc.compile()
    return nc


# ---------------------------------------------------------------- host side

def pack_inputs(x, y, Wq1, Wk1, Wv1, Wo1, Wq2, Wk2, Wv2, Wo2,
                W_in, b_in, W_out, b_out):
    NH = H // 2

    def tr_bf16(a):            # [T, D] -> [128, ND, T]
        return np.ascontiguousarray(
            a.T.reshape(ND, P, T).transpose(1, 0, 2)).astype(NPBF16)

    def qk_pack(W, h0):        # [H,D,DK] -> [128, ND, 512] pair-blocked
        Wh = W[h0:h0 + NH]
        Wp = Wh.reshape(NH // 2, 2, D, DK).transpose(2, 0, 1, 3)
        Wp = Wp.reshape(D, NH * DK)
        return np.ascontiguousarray(
            Wp.reshape(ND, P, NH * DK).transpose(1, 0, 2)).astype(NPBF16)

    def v_pack(W, h0):
        Wh = W[h0:h0 + NH].transpose(1, 0, 2).reshape(D, NH * DV)
        return np.ascontiguousarray(
            Wh.reshape(ND, P, NH * DV).transpose(1, 0, 2)).astype(NPBF16)

    def wo_pack(Wo, h):        # my half rows of Wo -> [128, 4, D]
        Ws = Wo[NH * DV * h:NH * DV * (h + 1)] * (RS_SCALE / (WV_SCALE * WVP_BOOST))
        return np.ascontiguousarray(
            Ws.reshape(4, P, D).transpose(1, 0, 2)).astype(NPBF16)

    def wi_pack(W):            # [FF, D] -> [NF, 128, ND, 128]
        A = W.T.reshape(ND, P, NF, P)
        return np.ascontiguousarray(A.transpose(2, 1, 0, 3)).astype(NPBF16)

    def wot_pack(W):           # [D, FF] -> [NF, 128, D]
        return np.ascontiguousarray(
            W.T.reshape(NF, P, D)).astype(NPBF16)

    # scores need /sqrt(DK) = 1/8 total; split sqrt evenly across Q and K
    # packs so both stay in a healthy fp8 range.
    sq = np.float32(1.0 / np.sqrt(np.sqrt(np.float32(DK))))

    wi_p = wi_pack(np.asarray(W_in))
    wot_p = wot_pack(np.asarray(W_out))
    bi_p = np.ascontiguousarray(
        np.asarray(b_in).reshape(NF, P).T).astype(np.float32)

    in_maps = []
    for c in range(2 * x.shape[0]):
        b, h = c // 2, c % 2
        h0 = NH * h
        yb, xb = y[b], x[b]
        yTo_full = tr_bf16(yb)      # [128, ND, T]
        in_maps.append(dict(
            yT=yTo_full,
            xT=tr_bf16(xb),
            yTo=np.ascontiguousarray(yTo_full[:, :, h * TH:(h + 1) * TH]),
            ynb=(np.asarray(yb[h * TH:(h + 1) * TH]) +
                 np.asarray(b_out)[None, :]).astype(np.float32),
            wq1=qk_pack(Wq1 * sq, h0), wk1=qk_pack(Wk1 * sq, h0),
            wv1=v_pack(Wv1 * WV_SCALE, h0), wo1=wo_pack(np.asarray(Wo1), h),
            wq2=qk_pack(Wq2 * sq, h0), wk2=qk_pack(Wk2 * sq, h0),
            wv2=v_pack(Wv2 * WV_SCALE, h0), wo2=wo_pack(np.asarray(Wo2), h),
            wi=wi_p, wot=wot_p, bi=bi_p,
        ))
    return in_maps


_PROG_CACHE = {}


def kernel(**inputs) -> np.ndarray:
    inputs = {k: np.asarray(v, np.float32) for k, v in inputs.items()}
    if "full" not in _PROG_CACHE:
        _PROG_CACHE["full"] = build_program()
    nc = _PROG_CACHE["full"]
    in_maps = pack_inputs(**inputs)
    res = run_bass_kernel_spmd(nc, in_maps, core_ids=list(range(8)))
    out = np.empty((B, T, D), np.float32)
    for c in range(8):
        b, h = c // 2, c % 2
        out[b, h * TH:(h + 1) * TH] = res.results[c]["out"]
    return out



# revision 11
# speedup vs baseline: 1.1259x; 1.1259x over previous
"""Trainium2 Bass kernel for nn_DecoderStack — v4.

Changes vs v3 (469us):
  * fp8 DoubleRow matmuls (2x PE throughput, measured on hw) for the QKV
    projections and the Wo applications. Weights are rescaled into fp8's
    normal range (Wq/Wk x16*sq with exp(scale=1/256); Wo x4; V x48) so
    quantization stays ~6%/element instead of drowning in subnormals.
  * The two ReduceScatters stay (pair-rank routing needs a sum collective;
    AllToAll is mesh-only) but are rescheduled: Wo units are ~3x cheaper
    (DoubleRow over my 512 Wo rows), so RS1 triggers ~15us into heads2 and
    its m-adds sit at the very END of heads2's program order -> no vector
    head-of-line stall (v3 lost ~18us + a HAM down-clock there). RS2 hides
    under the first FFN ffp sweep.
  * FFN stays bf16 (fp8 there measurably blows the 2e-2 error budget).
  * pt1/pt2 get distinct buffers (v3 aliased them -> heads2 partials
    serialized behind wo1's pt1 reads).
"""

import sys

for _p in ("/opt/trn_rl_repo", "/root/.axon_site"):
    if _p not in sys.path:
        sys.path.insert(0, _p)

import contextlib

import numpy as np

import concourse.bass as bass
import concourse.bacc as bacc
import concourse.tile as tile
from concourse import mybir
from concourse.bass_utils import run_bass_kernel_spmd

B, T, D, H, DK, DV, FF = 4, 1024, 1024, 16, 64, 64, 4096
P = 128
TH = T // 2           # rows owned per core
NT = T // P           # 8 t/s tiles over full T
ND = D // P           # 8 d chunks
NF = FF // P          # 32 ff chunks
NTO = TH // P         # 4 own-row tiles
FP32 = mybir.dt.float32
BF16 = mybir.dt.bfloat16
FP8 = mybir.dt.float8e4
DR = mybir.MatmulPerfMode.DoubleRow
NPBF16 = mybir.dt.np(BF16)
QK_SCALE = 16.0       # per-side boost on Wq/Wk packs (scores x256 -> exp scale)
WV_SCALE = 48.0       # fp8: wv*48 keeps |V| < 240
WVP_BOOST = 8.0       # extra boost on wvp (= wva/denom) to avoid denormals
WO_SCALE = 4.0        # Wo pack boost into fp8 normal range
RS_SCALE = 16.0       # m-branch RS payload in fp8 at 16x scale
M_PSUM = WV_SCALE * WVP_BOOST * WO_SCALE   # wo psum = M_PSUM * m


def build_program(n_cores: int = 8, compile: bool = True):
    nc = bacc.Bacc("TRN2", target_bir_lowering=False, debug=False,
                   num_devices=n_cores)
    groups = [[2 * g, 2 * g + 1] for g in range(n_cores // 2)]

    def dram_in(name, shape, dt=FP8):
        return nc.dram_tensor(name, shape, dt, kind="ExternalInput")

    yT = dram_in("yT", [P, ND, T])
    xT = dram_in("xT", [P, ND, T])
    yTo = dram_in("yTo", [P, ND, TH], BF16)
    ynb = dram_in("ynb", [TH, D], FP32)        # y own rows + b_out
    wq1 = dram_in("wq1", [P, ND, 512])
    wk1 = dram_in("wk1", [P, ND, 512])
    wv1 = dram_in("wv1", [P, ND, 512])
    wo1 = dram_in("wo1", [P, 4, D])
    wq2 = dram_in("wq2", [P, ND, 512])
    wk2 = dram_in("wk2", [P, ND, 512])
    wv2 = dram_in("wv2", [P, ND, 512])
    wo2 = dram_in("wo2", [P, 4, D])
    wi = dram_in("wi", [NF, P, ND, P], BF16)
    wot = dram_in("wot", [NF, P, D], BF16)
    bi = dram_in("bi", [P, NF], FP32)
    out = nc.dram_tensor("out", [TH, D], FP32, kind="ExternalOutput")

    with tile.TileContext(nc) as tc:
        with contextlib.ExitStack() as ctx:
            p1 = ctx.enter_context(tc.tile_pool(name="p1", bufs=1))
            expp = ctx.enter_context(tc.tile_pool(name="expp", bufs=20))
            small = ctx.enter_context(tc.tile_pool(name="small", bufs=2))
            psum = ctx.enter_context(tc.tile_pool(name="psum", bufs=2, space="PSUM"))
            dram = ctx.enter_context(tc.tile_pool(name="dram", bufs=1, space="DRAM"))

            rs1_in = dram.tile([T, D], FP8, tag="rs1i", name="rs1_in")
            rs1_out = dram.tile([TH, D], FP8, tag="rs1o", name="rs1_out")
            rs2_in = dram.tile([T, D], FP8, tag="rs2i", name="rs2_in")
            rs2_out = dram.tile([TH, D], FP8, tag="rs2o", name="rs2_out")

            # ---------------- warmup (HAM) + persistent loads
            junk = p1.tile([P, 512], BF16, tag="junk")
            nc.gpsimd.memset(junk[:], 0.25)
            jp = [psum.tile([P, 512], FP32, tag="st", name=f"jp{i}")
                  for i in range(2)]
            for i in range(44):
                nc.tensor.matmul(jp[i % 2][:], lhsT=junk[:, 0:P], rhs=junk[:],
                                 start=True, stop=True, skip_group_check=True)

            yT_sb = p1.tile([P, ND, T], FP8, tag="yT")
            for dc in range(ND):
                nc.sync.dma_start(yT_sb[:, dc, :], yT[:, dc, :])
            wq_sb = p1.tile([P, ND, 512], FP8, tag="wq", name="wq1_sb")
            wk_sb = p1.tile([P, ND, 512], FP8, tag="wk", name="wk1_sb")
            nc.sync.dma_start(wq_sb[:], wq1[:])
            nc.sync.dma_start(wk_sb[:], wk1[:])
            S = p1.tile([P, NTO, D], FP32, tag="S")
            for t_ in range(NTO):
                nc.sync.dma_start(S[:, t_, :], ynb[t_ * P:(t_ + 1) * P, :])
            wv_sb = p1.tile([P, ND, 512], FP8, tag="wv", name="wv1_sb")
            nc.sync.dma_start(wv_sb[:], wv1[:])
            xT_sb = p1.tile([P, ND, T], FP8, tag="xT")
            for dc in range(ND):
                nc.sync.dma_start(xT_sb[:, dc, :], xT[:, dc, :])
            yTo_sb = p1.tile([P, ND, TH], BF16, tag="yTo")
            nc.sync.dma_start(yTo_sb[:], yTo[:])
            bi_sb = p1.tile([P, NF], FP32, tag="bi")
            nc.sync.dma_start(bi_sb[:], bi[:])

            # ---------------- building blocks
            def qk_pair(wsb, dst, p, rhs_sb):
                """Project one 128-col block (head pair p) of Q or K via
                DoubleRow fp8. Uses the 2-bank "sc" psum; only OUTSIDE
                head loops."""
                ps = psum.tile([P, T], FP32, tag="sc", name=f"qk_{dst.name}_{p}")
                for dcp in range(ND // 2):
                    for th in range(2):
                        nc.tensor.matmul(
                            ps[:, th * TH:(th + 1) * TH],
                            lhsT=wsb[:, 2 * dcp:2 * dcp + 2, p * P:(p + 1) * P],
                            rhs=rhs_sb[:, 2 * dcp:2 * dcp + 2,
                                       th * TH:(th + 1) * TH],
                            start=(dcp == 0), stop=(dcp == ND // 2 - 1),
                            perf_mode=DR, skip_group_check=True)
                nc.vector.tensor_copy(out=dst[:, p, :], in_=ps[:])

            def qk_half(wsb, dst, p, th, rhs_sb):
                """Stuffable DR half projection using a 1-bank "st" tile."""
                ps = psum.tile([P, TH], FP32, tag="st",
                               name=f"qkh_{dst.name}_{p}_{th}")
                for dcp in range(ND // 2):
                    nc.tensor.matmul(
                        ps[:],
                        lhsT=wsb[:, 2 * dcp:2 * dcp + 2, p * P:(p + 1) * P],
                        rhs=rhs_sb[:, 2 * dcp:2 * dcp + 2,
                                   th * TH:(th + 1) * TH],
                        start=(dcp == 0), stop=(dcp == ND // 2 - 1),
                        perf_mode=DR)
                nc.vector.tensor_copy(out=dst[:, p, th * TH:(th + 1) * TH],
                                      in_=ps[:])

            def v_unit(wva, st, kv_sb):
                pv = psum.tile([P, 512], FP32, tag="st", name=f"v_{wva.name}_{st}")
                for dcp in range(ND // 2):
                    nc.tensor.matmul(
                        pv[:],
                        lhsT=kv_sb[:, 2 * dcp:2 * dcp + 2,
                                   st * P:(st + 1) * P],
                        rhs=wv_sb[:, 2 * dcp:2 * dcp + 2, :],
                        start=(dcp == 0), stop=(dcp == ND // 2 - 1),
                        perf_mode=DR)
                nc.vector.tensor_copy(out=wva[:, st, :], in_=pv[:])

            def h1_unit(h1_sb, fc):
                wi_c = p1.tile([P, ND, P], BF16, tag="wic", bufs=3,
                               name=f"wic_{fc}")
                nc.sync.dma_start(wi_c[:], wi[fc])
                ph = psum.tile([P, TH], FP32, tag="st", name=f"h1_{fc}")
                for dc in range(ND):
                    nc.tensor.matmul(
                        ph[:], lhsT=wi_c[:, dc, :], rhs=yTo_sb[:, dc, :],
                        start=(dc == 0), stop=(dc == ND - 1))
                nc.vector.tensor_scalar(
                    out=h1_sb[:, fc, :], in0=ph[:],
                    scalar1=bi_sb[:, fc:fc + 1], scalar2=0.0,
                    op0=mybir.AluOpType.add, op1=mybir.AluOpType.max)

            def wo_unit(pt, wof, tt8, dh, dst_ap):
                """m-partial rows tt8 (full T), d-half dh -> fp8 -> DRAM rs
                buf. DoubleRow over my 512 Wo rows (2 chunk-pairs)."""
                ps = psum.tile([P, TH], FP32, tag="st",
                               name=f"wo_{wof.name}_{tt8}_{dh}")
                for cp in range(2):
                    nc.tensor.matmul(
                        ps[:],
                        lhsT=pt[:, 2 * cp:2 * cp + 2, tt8 * P:(tt8 + 1) * P],
                        rhs=wof[:, 2 * cp:2 * cp + 2, dh * TH:(dh + 1) * TH],
                        start=(cp == 0), stop=(cp == 1),
                        perf_mode=DR)
                stg = p1.tile([P, TH], FP8, tag="mstg", bufs=3,
                              name=f"mstg_{wof.name}_{tt8}_{dh}")
                nc.vector.tensor_scalar(
                    out=stg[:], in0=ps[:], scalar1=RS_SCALE / M_PSUM,
                    scalar2=0.0, op0=mybir.AluOpType.mult,
                    op1=mybir.AluOpType.bypass)
                nc.sync.dma_start(dst_ap, stg[:])

            def m_add(rs_out_ap, tt, tag):
                """S[tt] += rs_out / RS_SCALE (one own-row tile). The load
                rides the gpsimd DMA queue so the pending collective does
                not head-of-line-block the sync-engine DMA stream."""
                mld = p1.tile([P, D], FP8, tag="mld", bufs=4,
                              name=f"mld_{tag}_{tt}")
                nc.gpsimd.dma_start(mld[:], rs_out_ap)
                nc.vector.scalar_tensor_tensor(
                    out=S[:, tt, :], in0=mld[:], scalar=1.0 / RS_SCALE,
                    in1=S[:, tt, :],
                    op0=mybir.AluOpType.mult, op1=mybir.AluOpType.add)

            # stuffing queue machinery
            stuff_q = []

            def maybe_stuff(n=1):
                for _ in range(n):
                    if stuff_q:
                        stuff_q.pop(0)()

            def heads(m, wqt, wkt, wva, pt):
                """4 head-pairs; scores row-group paired; exp fp8; partial."""
                prev = None

                def partial_block(p, exA, exB, den):
                    rden = small.tile([P, 2, NT], FP32, tag="rden",
                                      name=f"rden{m}_{p}")
                    nc.vector.reciprocal(out=rden[:], in_=den[:])
                    wvp = small.tile([P, 2, NT, DV], FP8, tag="wvp",
                                     name=f"wvp{m}_{p}")
                    for j in range(2):
                        for st in range(NT):
                            nc.vector.tensor_scalar(
                                out=wvp[:, j, st, :],
                                in0=wva[:, st, (2 * p + j) * DV:(2 * p + j + 1) * DV],
                                scalar1=rden[:, j, st:st + 1],
                                scalar2=WVP_BOOST,
                                op0=mybir.AluOpType.mult,
                                op1=mybir.AluOpType.mult)
                    pps = [psum.tile([P, TH], FP32, tag="pp", name=f"pp{m}_{p}_{j}")
                           for j in range(2)]
                    for j, ex in ((0, exA), (1, exB)):
                        for st in range(NT):
                            nc.tensor.matmul(
                                pps[j][0:64, :], lhsT=wvp[:, j, st, :],
                                rhs=ex[st][:, 0:TH],
                                start=(st == 0), stop=(st == NT - 1),
                                skip_group_check=True)
                            nc.tensor.matmul(
                                pps[j][64:128, :], lhsT=wvp[:, j, st, :],
                                rhs=ex[st][:, TH:T],
                                start=(st == 0), stop=(st == NT - 1),
                                tile_position=(0, 64), skip_group_check=True)
                    for j in range(2):
                        lo, hi = 64 * j, 64 * j + 64
                        nc.vector.tensor_copy(out=pt[lo:hi, p, 0:TH],
                                              in_=pps[j][0:64, :])
                        nc.vector.tensor_copy(out=pt[lo:hi, p, TH:T],
                                              in_=pps[j][64:128, :])

                for p in range(4):
                    den = small.tile([P, 2, NT], FP32, tag="den",
                                     name=f"den{m}_{p}")
                    exA, exB = [], []
                    for st in range(NT):
                        psA = psum.tile([P, T], FP32, tag="sc",
                                        name=f"scA{m}_{p}_{st}")
                        psB = psum.tile([P, T], FP32, tag="sc",
                                        name=f"scB{m}_{p}_{st}")
                        for th in range(2):
                            tsl = slice(th * TH, (th + 1) * TH)
                            nc.tensor.matmul(
                                psA[:, tsl],
                                lhsT=wkt[0:64, p, st * P:(st + 1) * P],
                                rhs=wqt[0:64, p, tsl],
                                start=True, stop=True, skip_group_check=True)
                            nc.tensor.matmul(
                                psB[:, tsl],
                                lhsT=wkt[64:128, p, st * P:(st + 1) * P],
                                rhs=wqt[64:128, p, tsl],
                                start=True, stop=True, skip_group_check=True)
                        eA = expp.tile([P, T], FP8, tag="exp",
                                       name=f"exA{m}_{p}_{st}")
                        nc.scalar.activation(
                            out=eA[:], in_=psA[:],
                            func=mybir.ActivationFunctionType.Exp,
                            scale=1.0 / (QK_SCALE * QK_SCALE),
                            accum_out=den[:, 0, st:st + 1])
                        eB = expp.tile([P, T], FP8, tag="exp",
                                       name=f"exB{m}_{p}_{st}")
                        nc.scalar.activation(
                            out=eB[:], in_=psB[:],
                            func=mybir.ActivationFunctionType.Exp,
                            scale=1.0 / (QK_SCALE * QK_SCALE),
                            accum_out=den[:, 1, st:st + 1])
                        exA.append(eA)
                        exB.append(eB)
                        if st < 7:
                            maybe_stuff(1)
                    if prev is not None:
                        partial_block(*prev)
                        maybe_stuff(3)
                    prev = (p, exA, exB, den)
                partial_block(*prev)
                maybe_stuff(3)

            # ---------------- QKV1 projections (PE dense from the start)
            wqt1 = p1.tile([P, 4, T], FP8, tag="wqt", bufs=2, name="wqt1")
            wkt1 = p1.tile([P, 4, T], FP8, tag="wkt", bufs=2, name="wkt1")
            for p in range(4):
                qk_pair(wq_sb, wqt1, p, yT_sb)
                qk_pair(wk_sb, wkt1, p, yT_sb)

            wva1 = p1.tile([P, NT, 512], FP8, tag="wva", bufs=2, name="wva1")
            wva2 = p1.tile([P, NT, 512], FP8, tag="wva", bufs=2, name="wva2")
            h1_sb = p1.tile([P, NF, TH], BF16, tag="h1")

            wq2_sb = p1.tile([P, ND, 512], FP8, tag="wq", name="wq2_sb")
            wk2_sb = p1.tile([P, ND, 512], FP8, tag="wk", name="wk2_sb")
            wv2_sb = p1.tile([P, ND, 512], FP8, tag="wv", name="wv2_sb")
            wqt2 = p1.tile([P, 4, T], FP8, tag="wqt", bufs=2, name="wqt2")
            wkt2 = p1.tile([P, 4, T], FP8, tag="wkt", bufs=2, name="wkt2")

            def load_w2():
                nc.sync.dma_start(wq2_sb[:], wq2[:])
                nc.sync.dma_start(wk2_sb[:], wk2[:])

            def load_wv2():
                nc.sync.dma_start(wv2_sb[:], wv2[:])

            def qk2_units(p):
                return [lambda th=th: qk_half(wq2_sb, wqt2, p, th, yT_sb)
                        for th in range(2)] + \
                       [lambda th=th: qk_half(wk2_sb, wkt2, p, th, xT_sb)
                        for th in range(2)]

            # heads1 stuffing: v1 / v2 / qk2 p0-p3 / h1 (tail spills to heads2)
            for st in range(NT):
                stuff_q.append(lambda st=st: v_unit(wva1, st, yT_sb))
            stuff_q.append(load_w2)
            stuff_q.append(load_wv2)
            stuff_q.extend(qk2_units(0))
            for st in range(NT):
                stuff_q.append(lambda st=st: v_unit(wva2, st, xT_sb))
            stuff_q.extend(qk2_units(1))
            for fc in range(8):
                stuff_q.append(lambda fc=fc: h1_unit(h1_sb, fc))
            stuff_q.extend(qk2_units(2))
            stuff_q.extend(qk2_units(3))
            for fc in range(8, 16):
                stuff_q.append(lambda fc=fc: h1_unit(h1_sb, fc))

            # ---------------- heads1 (scalar-bound; stuffed)
            pt1 = p1.tile([P, 4, T], FP8, tag="pt", bufs=2, name="pt1")
            heads(1, wqt1, wkt1, wva1, pt1)

            # ---------------- m1 = pt1 @ Wo1 (full T) -> RS1. The wo1 units
            # go FIRST in the heads2 stuff queue (cheap with DR), so RS1
            # triggers ~15us into heads2; the m1 adds are the LAST stuffed
            # closures, giving the collective the whole phase to land.
            wo1f = p1.tile([P, 4, D], FP8, tag="wof", bufs=2, name="wo1f")
            nc.sync.dma_start(wo1f[:], wo1[:])
            wo2f = p1.tile([P, 4, D], FP8, tag="wof", bufs=2, name="wo2f")
            nc.sync.dma_start(wo2f[:], wo2[:])

            for tt8 in range(NT):
                for dh in range(2):
                    stuff_q.append(lambda tt8=tt8, dh=dh: wo_unit(
                        pt1, wo1f, tt8, dh,
                        rs1_in[tt8 * P:(tt8 + 1) * P, dh * TH:(dh + 1) * TH]))

            def rs1_cc():
                nc.gpsimd.collective_compute(
                    "ReduceScatter", mybir.AluOpType.add, replica_groups=groups,
                    ins=[rs1_in.opt()], outs=[rs1_out.opt()])

            stuff_q.append(rs1_cc)
            for fc in range(16, NF):
                stuff_q.append(lambda fc=fc: h1_unit(h1_sb, fc))
            for tt in range(NTO):
                stuff_q.append(lambda tt=tt: m_add(
                    rs1_out[tt * P:(tt + 1) * P, :], tt, "m1"))

            # ---------------- heads2
            pt2 = p1.tile([P, 4, T], FP8, tag="pt", bufs=2, name="pt2")
            heads(2, wqt2, wkt2, wva2, pt2)
            while stuff_q:
                maybe_stuff(1)

            # ---------------- m2 -> RS2 (hidden under the first ffp sweep)
            for tt8 in range(NT):
                for dh in range(2):
                    wo_unit(pt2, wo2f, tt8, dh,
                            rs2_in[tt8 * P:(tt8 + 1) * P,
                                   dh * TH:(dh + 1) * TH])
            nc.gpsimd.collective_compute(
                "ReduceScatter", mybir.AluOpType.add, replica_groups=groups,
                ins=[rs2_in.opt()], outs=[rs2_out.opt()])

            def final_chain(tt):
                stats = small.tile([P, 2, 6], FP32, tag="stats",
                                   name=f"stats_{tt}")
                for i in range(2):
                    nc.vector.bn_stats(out=stats[:, i, :],
                                       in_=S[:, tt, i * TH:(i + 1) * TH])
                mv = small.tile([P, 2], FP32, tag="mv", name=f"mv_{tt}")
                nc.vector.bn_aggr(out=mv[:], in_=stats[:])
                std = small.tile([P, 1], FP32, tag="std", name=f"std_{tt}")
                nc.scalar.activation(
                    out=std[:], in_=mv[:, 1:2],
                    func=mybir.ActivationFunctionType.Sqrt,
                    scale=float(D) / float(D - 1))
                msum = small.tile([P, 1], FP32, tag="msum", name=f"msum_{tt}")
                nc.vector.tensor_add(out=msum[:], in0=mv[:, 0:1], in1=std[:])
                nc.vector.tensor_scalar_sub(out=S[:, tt, :], in0=S[:, tt, :],
                                            scalar1=msum[:])
                nc.sync.dma_start(out[tt * P:(tt + 1) * P, :], S[:, tt, :])

            # ffp sweep over a tt pair; S += acc at the end
            def ffp_sweep(tts):
                acc = [psum.tile([P, T], FP32, tag="sc", name=f"ffa_{tts[0]}_{k}")
                       for k in range(2)]
                for fc in range(NF):
                    wot_c = p1.tile([P, D], BF16, tag="wotc", bufs=4,
                                    name=f"wotc_{tts[0]}_{fc}")
                    nc.sync.dma_start(wot_c[:], wot[fc])
                    for k, tt in enumerate(tts):
                        for dh in range(2):
                            nc.tensor.matmul(
                                acc[k][:, dh * TH:(dh + 1) * TH],
                                lhsT=h1_sb[:, fc, tt * P:(tt + 1) * P],
                                rhs=wot_c[:, dh * TH:(dh + 1) * TH],
                                start=(fc == 0), stop=(fc == NF - 1),
                                skip_group_check=True)
                for k, tt in enumerate(tts):
                    for dh in range(2):
                        nc.vector.tensor_add(
                            out=S[:, tt, dh * TH:(dh + 1) * TH],
                            in0=acc[k][:, dh * TH:(dh + 1) * TH],
                            in1=S[:, tt, dh * TH:(dh + 1) * TH])

            # ---------------- endgame
            ffp_sweep((0, 1))
            m_add(rs2_out[0 * P:1 * P, :], 0, "m2")
            m_add(rs2_out[1 * P:2 * P, :], 1, "m2")
            final_chain(0)
            final_chain(1)
            ffp_sweep((2, 3))
            m_add(rs2_out[2 * P:3 * P, :], 2, "m2")
            m_add(rs2_out[3 * P:4 * P, :], 3, "m2")
            final_chain(2)
            final_chain(3)

    if compile:
        nc.compile()
    return nc


# ---------------------------------------------------------------- host side

def pack_inputs(x, y, Wq1, Wk1, Wv1, Wo1, Wq2, Wk2, Wv2, Wo2,
                W_in, b_in, W_out, b_out):
    NH = H // 2
    NPFP8 = mybir.dt.np(FP8)

    def q8(a):
        return np.clip(a, -240.0, 240.0).astype(NPFP8)

    def tr8(a):                # [T, D] -> fp8 [128, ND, T]
        return q8(np.ascontiguousarray(
            a.T.reshape(ND, P, T).transpose(1, 0, 2)))

    def trb(a):                # [T, D] -> bf16 [128, ND, T-slice]
        return np.ascontiguousarray(
            a.T.reshape(ND, P, a.shape[0]).transpose(1, 0, 2)).astype(NPBF16)

    def qk_pack(W, h0):        # [H,D,DK] -> fp8 [128, ND, 512] pair-blocked
        Wh = W[h0:h0 + NH]
        Wp = Wh.reshape(NH // 2, 2, D, DK).transpose(2, 0, 1, 3)
        Wp = Wp.reshape(D, NH * DK)
        return q8(np.ascontiguousarray(
            Wp.reshape(ND, P, NH * DK).transpose(1, 0, 2)))

    def v_pack(W, h0):
        Wh = W[h0:h0 + NH].transpose(1, 0, 2).reshape(D, NH * DV)
        return q8(np.ascontiguousarray(
            Wh.reshape(ND, P, NH * DV).transpose(1, 0, 2)))

    def wo_pack(Wo, h):        # my half rows of Wo -> fp8 [128, 4, D]
        Ws = Wo[NH * DV * h:NH * DV * (h + 1)] * WO_SCALE
        return q8(np.ascontiguousarray(
            Ws.reshape(4, P, D).transpose(1, 0, 2)))

    def wi_pack(W):            # [FF, D] -> bf16 [NF, 128, ND, 128]
        A = W.T.reshape(ND, P, NF, P)
        return np.ascontiguousarray(A.transpose(2, 1, 0, 3)).astype(NPBF16)

    def wot_pack(W):           # [D, FF] -> bf16 [NF, 128, D]
        return np.ascontiguousarray(
            W.T.reshape(NF, P, D)).astype(NPBF16)

    # scores need /sqrt(DK) total; QK_SCALE boost per side is undone by the
    # exp scale (1/QK_SCALE^2).
    sq = np.float32(QK_SCALE / np.sqrt(np.sqrt(np.float32(DK))))

    wi_p = wi_pack(np.asarray(W_in))
    wot_p = wot_pack(np.asarray(W_out))
    bi_p = np.ascontiguousarray(
        np.asarray(b_in).reshape(NF, P).T).astype(np.float32)

    in_maps = []
    for c in range(2 * x.shape[0]):
        b, h = c // 2, c % 2
        h0 = NH * h
        yb, xb = np.asarray(y[b]), np.asarray(x[b])
        in_maps.append(dict(
            yT=tr8(yb),
            xT=tr8(xb),
            yTo=trb(yb)[:, :, h * TH:(h + 1) * TH].copy(),
            ynb=(yb[h * TH:(h + 1) * TH] +
                 np.asarray(b_out)[None, :]).astype(np.float32),
            wq1=qk_pack(Wq1 * sq, h0), wk1=qk_pack(Wk1 * sq, h0),
            wv1=v_pack(Wv1 * WV_SCALE, h0), wo1=wo_pack(np.asarray(Wo1), h),
            wq2=qk_pack(Wq2 * sq, h0), wk2=qk_pack(Wk2 * sq, h0),
            wv2=v_pack(Wv2 * WV_SCALE, h0), wo2=wo_pack(np.asarray(Wo2), h),
            wi=wi_p, wot=wot_p, bi=bi_p,
        ))
    return in_maps


_PROG_CACHE = {}


def kernel(**inputs) -> np.ndarray:
    inputs = {k: np.asarray(v, np.float32) for k, v in inputs.items()}
    if "full" not in _PROG_CACHE:
        _PROG_CACHE["full"] = build_program()
    nc = _PROG_CACHE["full"]
    in_maps = pack_inputs(**inputs)
    res = run_bass_kernel_spmd(nc, in_maps, core_ids=list(range(8)))
    out = np.empty((B, T, D), np.float32)
    for c in range(8):
        b, h = c // 2, c % 2
        out[b, h * TH:(h + 1) * TH] = res.results[c]["out"]
    return out


# revision 61
# speedup vs baseline: 1.2012x; 1.0669x over previous
"""Trainium2 Bass kernel for nn_DecoderStack — v4.

Changes vs v3 (469us):
  * fp8 DoubleRow matmuls (2x PE throughput, measured on hw) for the QKV
    projections and the Wo applications. Weights are rescaled into fp8's
    normal range (Wq/Wk x16*sq with exp(scale=1/256); Wo x4; V x48) so
    quantization stays ~6%/element instead of drowning in subnormals.
  * The two ReduceScatters stay (pair-rank routing needs a sum collective;
    AllToAll is mesh-only <4 cores) but are rescheduled: wo1 units are ~3x
    cheaper (DoubleRow), stuffed first into heads2, so RS1 flies fully
    under heads2 (scalar-bound there, so the HAM CC-throttle is cheap);
    a tiny dummy RS at t=0 absorbs the ~11us first-cc stream spin-up.
    m1 adds moved to the endgame (their RS-gated vector STTs stalled the
    vector queue inside heads2 otherwise). RS2 hides under ffp sweep A.
  * FFN stays bf16 (fp8 there measurably blows the 2e-2 error budget).
  * pt1/pt2 distinct buffers; wo2 psum rotates st+pp tags (4 banks) and
    casts alternate vector/scalar so the post-heads2 chain to the RS2
    trigger is not vector-serialized; h1 relu for fc>=24 runs on scalar.
  * Measured 380-383us (from 469us baseline); rel err 1.354e-2.
    Known remaining costs: ~25us PE half-clock (HAM) while RS2 flies under
    ffp sweep A; ~180us scalar exp chain is the heads-phase floor; ~8us
    framework preamble. Failed experiments (all made it SLOWER, scheduler
    reorders defeat intuition): early-heads QKV1 split, cost-aware stuff
    budgeting, quiet-tail stuffing, all-scalar wo2 casts, m-adds after
    sweeps, junk warmup < 44, single-DMA yT/xT.
"""

import sys

for _p in ("/opt/trn_rl_repo", "/root/.axon_site"):
    if _p not in sys.path:
        sys.path.insert(0, _p)

import contextlib

import numpy as np

import concourse.bass as bass
import concourse.bacc as bacc
import concourse.tile as tile
from concourse import mybir
from concourse.bass_utils import run_bass_kernel_spmd

B, T, D, H, DK, DV, FF = 4, 1024, 1024, 16, 64, 64, 4096
P = 128
TH = T // 2           # rows owned per core
NT = T // P           # 8 t/s tiles over full T
ND = D // P           # 8 d chunks
NF = FF // P          # 32 ff chunks
NTO = TH // P         # 4 own-row tiles
FP32 = mybir.dt.float32
BF16 = mybir.dt.bfloat16
FP8 = mybir.dt.float8e4
DR = mybir.MatmulPerfMode.DoubleRow
NPBF16 = mybir.dt.np(BF16)
QK_SCALE = 16.0       # per-side boost on Wq/Wk packs (scores x256 -> exp scale)
WV_SCALE = 48.0       # fp8: wv*48 keeps |V| < 240
WVP_BOOST = 8.0       # extra boost on wvp (= wva/denom) to avoid denormals
WO_SCALE = 4.0        # Wo pack boost into fp8 normal range
RS_SCALE = 16.0       # m-branch RS payload in fp8 at 16x scale
M_PSUM = WV_SCALE * WVP_BOOST * WO_SCALE   # wo psum = M_PSUM * m


def build_program(n_cores: int = 8, compile: bool = True):
    nc = bacc.Bacc("TRN2", target_bir_lowering=False, debug=False,
                   num_devices=n_cores)
    groups = [[2 * g, 2 * g + 1] for g in range(n_cores // 2)]

    def dram_in(name, shape, dt=FP8):
        return nc.dram_tensor(name, shape, dt, kind="ExternalInput")

    yT = dram_in("yT", [P, ND, T])
    xT = dram_in("xT", [P, ND, T])
    yTo = dram_in("yTo", [P, ND, TH], BF16)
    ynb = dram_in("ynb", [TH, D], FP32)        # y own rows + b_out
    wq1 = dram_in("wq1", [P, ND, 512])
    wk1 = dram_in("wk1", [P, ND, 512])
    wv1 = dram_in("wv1", [P, ND, 512])
    wo1 = dram_in("wo1", [P, 4, D])
    wq2 = dram_in("wq2", [P, ND, 512])
    wk2 = dram_in("wk2", [P, ND, 512])
    wv2 = dram_in("wv2", [P, ND, 512])
    wo2 = dram_in("wo2", [P, 4, D])
    wi = dram_in("wi", [NF, P, ND, P], BF16)
    wot = dram_in("wot", [NF, P, D], BF16)
    bi = dram_in("bi", [P, NF], FP32)
    out = nc.dram_tensor("out", [TH, D], FP32, kind="ExternalOutput")

    with tile.TileContext(nc) as tc:
        with contextlib.ExitStack() as ctx:
            p1 = ctx.enter_context(tc.tile_pool(name="p1", bufs=1))
            expp = ctx.enter_context(tc.tile_pool(name="expp", bufs=20))
            small = ctx.enter_context(tc.tile_pool(name="small", bufs=2))
            psum = ctx.enter_context(tc.tile_pool(name="psum", bufs=2, space="PSUM"))
            dram = ctx.enter_context(tc.tile_pool(name="dram", bufs=1, space="DRAM"))

            rs1_in = dram.tile([T, D], FP8, tag="rs1i", name="rs1_in")
            rs1_out = dram.tile([TH, D], FP8, tag="rs1o", name="rs1_out")
            rs2_in = dram.tile([T, D], FP8, tag="rs2i", name="rs2_in")
            rs2_out = dram.tile([TH, D], FP8, tag="rs2o", name="rs2_out")
            rsd_in = dram.tile([2, 64], FP8, tag="rsdi", name="rsd_in")
            rsd_out = dram.tile([1, 64], FP8, tag="rsdo", name="rsd_out")

            # ---------------- warmup (HAM) + persistent loads
            junk = p1.tile([P, 512], BF16, tag="junk")
            nc.gpsimd.memset(junk[:], 0.25)
            # pre-load the Exp ACT table + scalar-engine preamble during the
            # input DMA phase so heads1's first real exp starts promptly
            ewarm = p1.tile([P, 8], FP32, tag="ewarm")
            nc.scalar.activation(out=ewarm[:], in_=junk[:, 0:8],
                                 func=mybir.ActivationFunctionType.Exp)
            jp = [psum.tile([P, 512], FP32, tag="st", name=f"jp{i}")
                  for i in range(2)]
            for i in range(44):
                nc.tensor.matmul(jp[i % 2][:], lhsT=junk[:, 0:P], rhs=junk[:],
                                 start=True, stop=True, skip_group_check=True)

            # tiny dummy collective: absorbs the ~11us first-cc stream
            # spin-up so RS1 starts promptly mid-heads2
            zt = p1.tile([2, 64], FP8, tag="zt")
            nc.gpsimd.memset(zt[:], 0.0)
            nc.gpsimd.dma_start(rsd_in[:], zt[:])
            nc.gpsimd.collective_compute(
                "ReduceScatter", mybir.AluOpType.add, replica_groups=groups,
                ins=[rsd_in.opt()], outs=[rsd_out.opt()])

            yT_sb = p1.tile([P, ND, T], FP8, tag="yT")
            for dc in range(ND):
                nc.sync.dma_start(yT_sb[:, dc, :], yT[:, dc, :])
            wq_sb = p1.tile([P, ND, 512], FP8, tag="wq", name="wq1_sb")
            wk_sb = p1.tile([P, ND, 512], FP8, tag="wk", name="wk1_sb")
            nc.sync.dma_start(wq_sb[:], wq1[:])
            nc.sync.dma_start(wk_sb[:], wk1[:])
            S = p1.tile([P, NTO, D], FP32, tag="S")
            for t_ in range(NTO):
                nc.sync.dma_start(S[:, t_, :], ynb[t_ * P:(t_ + 1) * P, :])
            wv_sb = p1.tile([P, ND, 512], FP8, tag="wv", name="wv1_sb")
            nc.sync.dma_start(wv_sb[:], wv1[:])
            xT_sb = p1.tile([P, ND, T], FP8, tag="xT")
            for dc in range(ND):
                nc.sync.dma_start(xT_sb[:, dc, :], xT[:, dc, :])
            yTo_sb = p1.tile([P, ND, TH], BF16, tag="yTo")
            nc.sync.dma_start(yTo_sb[:], yTo[:])
            bi_sb = p1.tile([P, NF], FP32, tag="bi")
            nc.sync.dma_start(bi_sb[:], bi[:])

            # ---------------- building blocks
            def qk_pair(wsb, dst, p, rhs_sb):
                """Project one 128-col block (head pair p) of Q or K via
                DoubleRow fp8. Uses the 2-bank "sc" psum; only OUTSIDE
                head loops."""
                ps = psum.tile([P, T], FP32, tag="sc", name=f"qk_{dst.name}_{p}")
                for dcp in range(ND // 2):
                    for th in range(2):
                        nc.tensor.matmul(
                            ps[:, th * TH:(th + 1) * TH],
                            lhsT=wsb[:, 2 * dcp:2 * dcp + 2, p * P:(p + 1) * P],
                            rhs=rhs_sb[:, 2 * dcp:2 * dcp + 2,
                                       th * TH:(th + 1) * TH],
                            start=(dcp == 0), stop=(dcp == ND // 2 - 1),
                            perf_mode=DR, skip_group_check=True)
                nc.vector.tensor_copy(out=dst[:, p, :], in_=ps[:])

            def qk_half(wsb, dst, p, th, rhs_sb):
                """Stuffable DR half projection using a 1-bank "st" tile."""
                ps = psum.tile([P, TH], FP32, tag="st",
                               name=f"qkh_{dst.name}_{p}_{th}")
                for dcp in range(ND // 2):
                    nc.tensor.matmul(
                        ps[:],
                        lhsT=wsb[:, 2 * dcp:2 * dcp + 2, p * P:(p + 1) * P],
                        rhs=rhs_sb[:, 2 * dcp:2 * dcp + 2,
                                   th * TH:(th + 1) * TH],
                        start=(dcp == 0), stop=(dcp == ND // 2 - 1),
                        perf_mode=DR)
                nc.vector.tensor_copy(out=dst[:, p, th * TH:(th + 1) * TH],
                                      in_=ps[:])

            def v_unit(wva, st, kv_sb):
                pv = psum.tile([P, 512], FP32, tag="st", name=f"v_{wva.name}_{st}")
                for dcp in range(ND // 2):
                    nc.tensor.matmul(
                        pv[:],
                        lhsT=kv_sb[:, 2 * dcp:2 * dcp + 2,
                                   st * P:(st + 1) * P],
                        rhs=wv_sb[:, 2 * dcp:2 * dcp + 2, :],
                        start=(dcp == 0), stop=(dcp == ND // 2 - 1),
                        perf_mode=DR)
                nc.vector.tensor_copy(out=wva[:, st, :], in_=pv[:])

            def h1_unit(h1_sb, fc):
                wi_c = p1.tile([P, ND, P], BF16, tag="wic", bufs=3,
                               name=f"wic_{fc}")
                nc.sync.dma_start(wi_c[:], wi[fc])
                ph = psum.tile([P, TH], FP32, tag="st", name=f"h1_{fc}")
                for dc in range(ND):
                    nc.tensor.matmul(
                        ph[:], lhsT=wi_c[:, dc, :], rhs=yTo_sb[:, dc, :],
                        start=(dc == 0), stop=(dc == ND - 1))
                if fc >= 24:
                    # late units land at the heads2 tail where vector is the
                    # serial bottleneck and scalar has gone idle
                    nc.scalar.activation(
                        out=h1_sb[:, fc, :], in_=ph[:],
                        func=mybir.ActivationFunctionType.Relu,
                        bias=bi_sb[:, fc:fc + 1])
                else:
                    nc.vector.tensor_scalar(
                        out=h1_sb[:, fc, :], in0=ph[:],
                        scalar1=bi_sb[:, fc:fc + 1], scalar2=0.0,
                        op0=mybir.AluOpType.add, op1=mybir.AluOpType.max)

            def wo_unit(pt, wof, tt8, dh, dst_ap, ptag="st", cast_eng="v"):
                """m-partial rows tt8 (full T), d-half dh -> fp8 -> DRAM rs
                buf. DoubleRow over my 512 Wo rows (2 chunk-pairs)."""
                ps = psum.tile([P, TH], FP32, tag=ptag,
                               name=f"wo_{wof.name}_{tt8}_{dh}")
                for cp in range(2):
                    nc.tensor.matmul(
                        ps[:],
                        lhsT=pt[:, 2 * cp:2 * cp + 2, tt8 * P:(tt8 + 1) * P],
                        rhs=wof[:, 2 * cp:2 * cp + 2, dh * TH:(dh + 1) * TH],
                        start=(cp == 0), stop=(cp == 1),
                        perf_mode=DR)
                stg = p1.tile([P, TH], FP8, tag="mstg", bufs=3,
                              name=f"mstg_{wof.name}_{tt8}_{dh}")
                if cast_eng == "s":
                    # scalar engine is idle post-heads2; split the casts so
                    # the wo2 chain is not vector-serialized
                    nc.scalar.activation(
                        out=stg[:], in_=ps[:],
                        func=mybir.ActivationFunctionType.Copy,
                        scale=RS_SCALE / M_PSUM)
                else:
                    nc.vector.tensor_scalar(
                        out=stg[:], in0=ps[:], scalar1=RS_SCALE / M_PSUM,
                        scalar2=0.0, op0=mybir.AluOpType.mult,
                        op1=mybir.AluOpType.bypass)
                nc.sync.dma_start(dst_ap, stg[:])

            def m_add(rs_out_ap, tt, tag):
                """S[tt] += rs_out / RS_SCALE (one own-row tile). Load AND
                add both ride gpsimd (idle), keeping the sync DMA stream and
                the vector engine free of collective-gated work."""
                mld = p1.tile([P, D], FP8, tag="mld", bufs=4,
                              name=f"mld_{tag}_{tt}")
                nc.gpsimd.dma_start(mld[:], rs_out_ap)
                nc.vector.scalar_tensor_tensor(
                    out=S[:, tt, :], in0=mld[:], scalar=1.0 / RS_SCALE,
                    in1=S[:, tt, :],
                    op0=mybir.AluOpType.mult, op1=mybir.AluOpType.add)

            # stuffing queue machinery
            stuff_q = []

            def maybe_stuff(n=1):
                for _ in range(n):
                    if stuff_q:
                        stuff_q.pop(0)()

            def heads(m, wqt, wkt, wva, pt):
                """4 head-pairs; scores row-group paired; exp fp8; partial."""
                prev = None

                def partial_block(p, exA, exB, den):
                    rden = small.tile([P, 2, NT], FP32, tag="rden",
                                      name=f"rden{m}_{p}")
                    nc.vector.reciprocal(out=rden[:], in_=den[:])
                    wvp = small.tile([P, 2, NT, DV], FP8, tag="wvp",
                                     name=f"wvp{m}_{p}")
                    for j in range(2):
                        for st in range(NT):
                            nc.vector.tensor_scalar(
                                out=wvp[:, j, st, :],
                                in0=wva[:, st, (2 * p + j) * DV:(2 * p + j + 1) * DV],
                                scalar1=rden[:, j, st:st + 1],
                                scalar2=WVP_BOOST,
                                op0=mybir.AluOpType.mult,
                                op1=mybir.AluOpType.mult)
                    pps = [psum.tile([P, TH], FP32, tag="pp", name=f"pp{m}_{p}_{j}")
                           for j in range(2)]
                    for j, ex in ((0, exA), (1, exB)):
                        for st in range(NT):
                            nc.tensor.matmul(
                                pps[j][0:64, :], lhsT=wvp[:, j, st, :],
                                rhs=ex[st][:, 0:TH],
                                start=(st == 0), stop=(st == NT - 1),
                                skip_group_check=True)
                            nc.tensor.matmul(
                                pps[j][64:128, :], lhsT=wvp[:, j, st, :],
                                rhs=ex[st][:, TH:T],
                                start=(st == 0), stop=(st == NT - 1),
                                tile_position=(0, 64), skip_group_check=True)
                    for j in range(2):
                        lo, hi = 64 * j, 64 * j + 64
                        nc.vector.tensor_copy(out=pt[lo:hi, p, 0:TH],
                                              in_=pps[j][0:64, :])
                        nc.vector.tensor_copy(out=pt[lo:hi, p, TH:T],
                                              in_=pps[j][64:128, :])

                for p in range(4):
                    den = small.tile([P, 2, NT], FP32, tag="den",
                                     name=f"den{m}_{p}")
                    exA, exB = [], []
                    for st in range(NT):
                        psA = psum.tile([P, T], FP32, tag="sc",
                                        name=f"scA{m}_{p}_{st}")
                        psB = psum.tile([P, T], FP32, tag="sc",
                                        name=f"scB{m}_{p}_{st}")
                        for th in range(2):
                            tsl = slice(th * TH, (th + 1) * TH)
                            nc.tensor.matmul(
                                psA[:, tsl],
                                lhsT=wkt[0:64, p, st * P:(st + 1) * P],
                                rhs=wqt[0:64, p, tsl],
                                start=True, stop=True, skip_group_check=True)
                            nc.tensor.matmul(
                                psB[:, tsl],
                                lhsT=wkt[64:128, p, st * P:(st + 1) * P],
                                rhs=wqt[64:128, p, tsl],
                                start=True, stop=True, skip_group_check=True)
                        eA = expp.tile([P, T], FP8, tag="exp",
                                       name=f"exA{m}_{p}_{st}")
                        nc.scalar.activation(
                            out=eA[:], in_=psA[:],
                            func=mybir.ActivationFunctionType.Exp,
                            scale=1.0 / (QK_SCALE * QK_SCALE),
                            accum_out=den[:, 0, st:st + 1])
                        eB = expp.tile([P, T], FP8, tag="exp",
                                       name=f"exB{m}_{p}_{st}")
                        nc.scalar.activation(
                            out=eB[:], in_=psB[:],
                            func=mybir.ActivationFunctionType.Exp,
                            scale=1.0 / (QK_SCALE * QK_SCALE),
                            accum_out=den[:, 1, st:st + 1])
                        exA.append(eA)
                        exB.append(eB)
                        if st < 7:
                            maybe_stuff(1)
                    if prev is not None:
                        partial_block(*prev)
                        maybe_stuff(3)
                    prev = (p, exA, exB, den)
                partial_block(*prev)
                maybe_stuff(3)

            # ---------------- QKV1 projections (PE dense from the start)
            wqt1 = p1.tile([P, 4, T], FP8, tag="wqt", bufs=2, name="wqt1")
            wkt1 = p1.tile([P, 4, T], FP8, tag="wkt", bufs=2, name="wkt1")
            for p in range(4):
                qk_pair(wq_sb, wqt1, p, yT_sb)
                qk_pair(wk_sb, wkt1, p, yT_sb)

            wva1 = p1.tile([P, NT, 512], FP8, tag="wva", bufs=2, name="wva1")
            wva2 = p1.tile([P, NT, 512], FP8, tag="wva", bufs=2, name="wva2")
            h1_sb = p1.tile([P, NF, TH], BF16, tag="h1")

            wq2_sb = p1.tile([P, ND, 512], FP8, tag="wq", name="wq2_sb")
            wk2_sb = p1.tile([P, ND, 512], FP8, tag="wk", name="wk2_sb")
            wv2_sb = p1.tile([P, ND, 512], FP8, tag="wv", name="wv2_sb")
            wqt2 = p1.tile([P, 4, T], FP8, tag="wqt", bufs=2, name="wqt2")
            wkt2 = p1.tile([P, 4, T], FP8, tag="wkt", bufs=2, name="wkt2")

            def load_w2():
                nc.sync.dma_start(wq2_sb[:], wq2[:])
                nc.sync.dma_start(wk2_sb[:], wk2[:])

            def load_wv2():
                nc.sync.dma_start(wv2_sb[:], wv2[:])

            def qk2_units(p):
                return [lambda th=th: qk_half(wq2_sb, wqt2, p, th, yT_sb)
                        for th in range(2)] + \
                       [lambda th=th: qk_half(wk2_sb, wkt2, p, th, xT_sb)
                        for th in range(2)]

            # heads1 stuffing: v1 / v2 / qk2 p0-p3 / h1 (tail spills to heads2)
            for st in range(NT):
                stuff_q.append(lambda st=st: v_unit(wva1, st, yT_sb))
            stuff_q.append(load_w2)
            stuff_q.append(load_wv2)
            stuff_q.extend(qk2_units(0))
            for st in range(NT):
                stuff_q.append(lambda st=st: v_unit(wva2, st, xT_sb))
            stuff_q.extend(qk2_units(1))
            for fc in range(8):
                stuff_q.append(lambda fc=fc: h1_unit(h1_sb, fc))
            stuff_q.extend(qk2_units(2))
            stuff_q.extend(qk2_units(3))
            for fc in range(8, 16):
                stuff_q.append(lambda fc=fc: h1_unit(h1_sb, fc))

            # ---------------- heads1 (scalar-bound; stuffed)
            pt1 = p1.tile([P, 4, T], FP8, tag="pt", bufs=2, name="pt1")
            heads(1, wqt1, wkt1, wva1, pt1)

            # ---------------- m1 = pt1 @ Wo1 (full T) -> RS1. The wo1 units
            # go FIRST in the heads2 stuff queue (cheap with DR), so RS1
            # triggers ~15us into heads2; the m1 adds are the LAST stuffed
            # closures, giving the collective the whole phase to land.
            wo1f = p1.tile([P, 4, D], FP8, tag="wof", bufs=2, name="wo1f")
            nc.sync.dma_start(wo1f[:], wo1[:])
            wo2f = p1.tile([P, 4, D], FP8, tag="wof", bufs=2, name="wo2f")
            nc.sync.dma_start(wo2f[:], wo2[:])

            for tt8 in range(NT):
                for dh in range(2):
                    stuff_q.append(lambda tt8=tt8, dh=dh: wo_unit(
                        pt1, wo1f, tt8, dh,
                        rs1_in[tt8 * P:(tt8 + 1) * P, dh * TH:(dh + 1) * TH]))

            def rs1_cc():
                nc.gpsimd.collective_compute(
                    "ReduceScatter", mybir.AluOpType.add, replica_groups=groups,
                    ins=[rs1_in.opt()], outs=[rs1_out.opt()])

            stuff_q.append(rs1_cc)
            for fc in range(16, NF):
                stuff_q.append(lambda fc=fc: h1_unit(h1_sb, fc))

            # ---------------- heads2
            pt2 = p1.tile([P, 4, T], FP8, tag="pt", bufs=2, name="pt2")
            heads(2, wqt2, wkt2, wva2, pt2)
            while stuff_q:
                maybe_stuff(1)

            # ---------------- m2 -> RS2 (hidden under the first ffp sweep).
            # Prefetch the first wot chunks so the sweep starts without
            # waiting behind the wo2 stg DMAs on the sync queue.
            wotc_pre = []
            for fc in range(4):
                wc = p1.tile([P, D], BF16, tag="wotc", bufs=4,
                             name=f"wotc_{fc}")
                nc.sync.dma_start(wc[:], wot[fc])
                wotc_pre.append(wc)
            for k, (tt8, dh) in enumerate((t8, d) for t8 in range(NT)
                                          for d in range(2)):
                wo_unit(pt2, wo2f, tt8, dh,
                        rs2_in[tt8 * P:(tt8 + 1) * P,
                               dh * TH:(dh + 1) * TH],
                        ptag=("st", "pp")[k % 2],
                        cast_eng=("v", "s")[k % 2])
            nc.gpsimd.collective_compute(
                "ReduceScatter", mybir.AluOpType.add, replica_groups=groups,
                ins=[rs2_in.opt()], outs=[rs2_out.opt()])

            def final_chain(tt):
                stats = small.tile([P, 2, 6], FP32, tag="stats",
                                   name=f"stats_{tt}")
                for i in range(2):
                    nc.vector.bn_stats(out=stats[:, i, :],
                                       in_=S[:, tt, i * TH:(i + 1) * TH])
                mv = small.tile([P, 2], FP32, tag="mv", name=f"mv_{tt}")
                nc.vector.bn_aggr(out=mv[:], in_=stats[:])
                std = small.tile([P, 1], FP32, tag="std", name=f"std_{tt}")
                nc.scalar.activation(
                    out=std[:], in_=mv[:, 1:2],
                    func=mybir.ActivationFunctionType.Sqrt,
                    scale=float(D) / float(D - 1))
                msum = small.tile([P, 1], FP32, tag="msum", name=f"msum_{tt}")
                nc.vector.tensor_add(out=msum[:], in0=mv[:, 0:1], in1=std[:])
                nc.vector.tensor_scalar_sub(out=S[:, tt, :], in0=S[:, tt, :],
                                            scalar1=msum[:])
                nc.sync.dma_start(out[tt * P:(tt + 1) * P, :], S[:, tt, :])

            # ---------------- endgame: m1 adds, then the ffp sweeps hide
            # RS2; m2 adds + final chains overlap the 2nd sweep.
            for tt in range(NTO):
                m_add(rs1_out[tt * P:(tt + 1) * P, :], tt, "m1")

            def ffp_sweep(tts):
                acc = [psum.tile([P, T], FP32, tag="sc", name=f"ffa_{tts[0]}_{k}")
                       for k in range(2)]
                for fc in range(NF):
                    if tts[0] == 0 and fc < 4:
                        wot_c = wotc_pre[fc]
                    else:
                        wot_c = p1.tile([P, D], BF16, tag="wotc", bufs=4,
                                        name=f"wotc_{tts[0]}_{fc}")
                        nc.sync.dma_start(wot_c[:], wot[fc])
                    for k, tt in enumerate(tts):
                        for dh in range(2):
                            nc.tensor.matmul(
                                acc[k][:, dh * TH:(dh + 1) * TH],
                                lhsT=h1_sb[:, fc, tt * P:(tt + 1) * P],
                                rhs=wot_c[:, dh * TH:(dh + 1) * TH],
                                start=(fc == 0), stop=(fc == NF - 1),
                                skip_group_check=True)
                for k, tt in enumerate(tts):
                    for dh in range(2):
                        nc.vector.tensor_add(
                            out=S[:, tt, dh * TH:(dh + 1) * TH],
                            in0=acc[k][:, dh * TH:(dh + 1) * TH],
                            in1=S[:, tt, dh * TH:(dh + 1) * TH])

            ffp_sweep((0, 1))
            m_add(rs2_out[0 * P:1 * P, :], 0, "m2")
            m_add(rs2_out[1 * P:2 * P, :], 1, "m2")
            final_chain(0)
            final_chain(1)
            ffp_sweep((2, 3))
            m_add(rs2_out[2 * P:3 * P, :], 2, "m2")
            m_add(rs2_out[3 * P:4 * P, :], 3, "m2")
            final_chain(2)
            final_chain(3)

    if compile:
        nc.compile()
    return nc


# ---------------------------------------------------------------- host side

def pack_inputs(x, y, Wq1, Wk1, Wv1, Wo1, Wq2, Wk2, Wv2, Wo2,
                W_in, b_in, W_out, b_out):
    NH = H // 2
    NPFP8 = mybir.dt.np(FP8)

    def q8(a):
        return np.clip(a, -240.0, 240.0).astype(NPFP8)

    def tr8(a):                # [T, D] -> fp8 [128, ND, T]
        return q8(np.ascontiguousarray(
            a.T.reshape(ND, P, T).transpose(1, 0, 2)))

    def trb(a):                # [T, D] -> bf16 [128, ND, T-slice]
        return np.ascontiguousarray(
            a.T.reshape(ND, P, a.shape[0]).transpose(1, 0, 2)).astype(NPBF16)

    def qk_pack(W, h0):        # [H,D,DK] -> fp8 [128, ND, 512] pair-blocked
        Wh = W[h0:h0 + NH]
        Wp = Wh.reshape(NH // 2, 2, D, DK).transpose(2, 0, 1, 3)
        Wp = Wp.reshape(D, NH * DK)
        return q8(np.ascontiguousarray(
            Wp.reshape(ND, P, NH * DK).transpose(1, 0, 2)))

    def v_pack(W, h0):
        Wh = W[h0:h0 + NH].transpose(1, 0, 2).reshape(D, NH * DV)
        return q8(np.ascontiguousarray(
            Wh.reshape(ND, P, NH * DV).transpose(1, 0, 2)))

    def wo_pack(Wo, h):        # my half rows of Wo -> fp8 [128, 4, D]
        Ws = Wo[NH * DV * h:NH * DV * (h + 1)] * WO_SCALE
        return q8(np.ascontiguousarray(
            Ws.reshape(4, P, D).transpose(1, 0, 2)))

    def wi_pack(W):            # [FF, D] -> bf16 [NF, 128, ND, 128]
        A = W.T.reshape(ND, P, NF, P)
        return np.ascontiguousarray(A.transpose(2, 1, 0, 3)).astype(NPBF16)

    def wot_pack(W):           # [D, FF] -> bf16 [NF, 128, D]
        return np.ascontiguousarray(
            W.T.reshape(NF, P, D)).astype(NPBF16)

    # scores need /sqrt(DK) total; QK_SCALE boost per side is undone by the
    # exp scale (1/QK_SCALE^2).
    sq = np.float32(QK_SCALE / np.sqrt(np.sqrt(np.float32(DK))))

    wi_p = wi_pack(np.asarray(W_in))
    wot_p = wot_pack(np.asarray(W_out))
    bi_p = np.ascontiguousarray(
        np.asarray(b_in).reshape(NF, P).T).astype(np.float32)

    in_maps = []
    for c in range(2 * x.shape[0]):
        b, h = c // 2, c % 2
        h0 = NH * h
        yb, xb = np.asarray(y[b]), np.asarray(x[b])
        in_maps.append(dict(
            yT=tr8(yb),
            xT=tr8(xb),
            yTo=trb(yb)[:, :, h * TH:(h + 1) * TH].copy(),
            ynb=(yb[h * TH:(h + 1) * TH] +
                 np.asarray(b_out)[None, :]).astype(np.float32),
            wq1=qk_pack(Wq1 * sq, h0), wk1=qk_pack(Wk1 * sq, h0),
            wv1=v_pack(Wv1 * WV_SCALE, h0), wo1=wo_pack(np.asarray(Wo1), h),
            wq2=qk_pack(Wq2 * sq, h0), wk2=qk_pack(Wk2 * sq, h0),
            wv2=v_pack(Wv2 * WV_SCALE, h0), wo2=wo_pack(np.asarray(Wo2), h),
            wi=wi_p, wot=wot_p, bi=bi_p,
        ))
    return in_maps


_PROG_CACHE = {}


def kernel(**inputs) -> np.ndarray:
    inputs = {k: np.asarray(v, np.float32) for k, v in inputs.items()}
    if "full" not in _PROG_CACHE:
        _PROG_CACHE["full"] = build_program()
    nc = _PROG_CACHE["full"]
    in_maps = pack_inputs(**inputs)
    res = run_bass_kernel_spmd(nc, in_maps, core_ids=list(range(8)))
    out = np.empty((B, T, D), np.float32)
    for c in range(8):
        b, h = c // 2, c % 2
        out[b, h * TH:(h + 1) * TH] = res.results[c]["out"]
    return out


# revision 73
# speedup vs baseline: 1.2716x; 1.0587x over previous
"""Trainium2 Bass kernel for nn_DecoderStack — v4.

Changes vs v3 (469us):
  * fp8 DoubleRow matmuls (2x PE throughput, measured on hw) for the QKV
    projections and the Wo applications. Weights are rescaled into fp8's
    normal range (Wq/Wk x16*sq with exp(scale=1/256); Wo x4; V x48) so
    quantization stays ~6%/element instead of drowning in subnormals.
  * The two ReduceScatters stay (pair-rank routing needs a sum collective;
    AllToAll is mesh-only <4 cores) but are rescheduled: wo1 units are ~3x
    cheaper (DoubleRow), stuffed first into heads2, so RS1 flies fully
    under heads2 (scalar-bound there, so the HAM CC-throttle is cheap);
    a tiny dummy RS at t=0 absorbs the ~11us first-cc stream spin-up.
    m1 adds moved to the endgame (their RS-gated vector STTs stalled the
    vector queue inside heads2 otherwise). RS2 hides under ffp sweep A.
  * FFN stays bf16 (fp8 there measurably blows the 2e-2 error budget).
  * pt1/pt2 distinct buffers; wo2 psum rotates st+pp tags (4 banks) and
    casts alternate vector/scalar so the post-heads2 chain to the RS2
    trigger is not vector-serialized; h1 relu for fc>=24 runs on scalar.
  * Measured 360-367us typical, best 360.2 (from 469us baseline); rel
    err 1.354e-2. Occasional runs land ~395-400 when the pair
    collectives run slow (inter-core skew / neighbor drift) - treat
    single-run deltas <15us as noise.
    Final wins: heads1 skips post-loop stuffing (final_stuff=False) so
    heads2's first scores aren't behind ~10us of h1 units; junk warmup
    24; ALL m-adds ride after/under the ffp sweeps (an m-add issued
    while its RS is in flight head-of-line-blocks the vector queue and
    starves wo2's psum rotation); wq/wk DMAs before yT chunks; tt2/3
    m-adds hoisted before sweep B.
    Known remaining costs: ~12us PE half-clock (HAM) while RS2 flies
    under ffp sweep A; ~180us scalar exp chain is the heads floor; ~8us
    framework preamble; ~30us ramp to first exp. Failed experiments
    (scheduler reorders defeat intuition - A/B everything, and re-run
    regressions before reverting): early-heads QKV1 split, cost-aware
    stuff budgeting, all-scalar wo2 casts, junk < 20, single-DMA
    yT/xT, wotc bufs=8 + m1(0,1) before sweep A.
"""

import sys

for _p in ("/opt/trn_rl_repo", "/root/.axon_site"):
    if _p not in sys.path:
        sys.path.insert(0, _p)

import contextlib

import numpy as np

import concourse.bass as bass
import concourse.bacc as bacc
import concourse.tile as tile
from concourse import mybir
from concourse.bass_utils import run_bass_kernel_spmd

B, T, D, H, DK, DV, FF = 4, 1024, 1024, 16, 64, 64, 4096
P = 128
TH = T // 2           # rows owned per core
NT = T // P           # 8 t/s tiles over full T
ND = D // P           # 8 d chunks
NF = FF // P          # 32 ff chunks
NTO = TH // P         # 4 own-row tiles
FP32 = mybir.dt.float32
BF16 = mybir.dt.bfloat16
FP8 = mybir.dt.float8e4
DR = mybir.MatmulPerfMode.DoubleRow
NPBF16 = mybir.dt.np(BF16)
QK_SCALE = 16.0       # per-side boost on Wq/Wk packs (scores x256 -> exp scale)
WV_SCALE = 48.0       # fp8: wv*48 keeps |V| < 240
WVP_BOOST = 8.0       # extra boost on wvp (= wva/denom) to avoid denormals
WO_SCALE = 4.0        # Wo pack boost into fp8 normal range
RS_SCALE = 16.0       # m-branch RS payload in fp8 at 16x scale
M_PSUM = WV_SCALE * WVP_BOOST * WO_SCALE   # wo psum = M_PSUM * m


def build_program(n_cores: int = 8, compile: bool = True):
    nc = bacc.Bacc("TRN2", target_bir_lowering=False, debug=False,
                   num_devices=n_cores)
    groups = [[2 * g, 2 * g + 1] for g in range(n_cores // 2)]

    def dram_in(name, shape, dt=FP8):
        return nc.dram_tensor(name, shape, dt, kind="ExternalInput")

    yT = dram_in("yT", [P, ND, T])
    xT = dram_in("xT", [P, ND, T])
    yTo = dram_in("yTo", [P, ND, TH], BF16)
    ynb = dram_in("ynb", [TH, D], FP32)        # y own rows + b_out
    wq1 = dram_in("wq1", [P, ND, 512])
    wk1 = dram_in("wk1", [P, ND, 512])
    wv1 = dram_in("wv1", [P, ND, 512])
    wo1 = dram_in("wo1", [P, 4, D])
    wq2 = dram_in("wq2", [P, ND, 512])
    wk2 = dram_in("wk2", [P, ND, 512])
    wv2 = dram_in("wv2", [P, ND, 512])
    wo2 = dram_in("wo2", [P, 4, D])
    wi = dram_in("wi", [NF, P, ND, P], BF16)
    wot = dram_in("wot", [NF, P, D], BF16)
    bi = dram_in("bi", [P, NF], FP32)
    out = nc.dram_tensor("out", [TH, D], FP32, kind="ExternalOutput")

    with tile.TileContext(nc) as tc:
        with contextlib.ExitStack() as ctx:
            p1 = ctx.enter_context(tc.tile_pool(name="p1", bufs=1))
            expp = ctx.enter_context(tc.tile_pool(name="expp", bufs=20))
            small = ctx.enter_context(tc.tile_pool(name="small", bufs=2))
            psum = ctx.enter_context(tc.tile_pool(name="psum", bufs=2, space="PSUM"))
            dram = ctx.enter_context(tc.tile_pool(name="dram", bufs=1, space="DRAM"))

            rs1_in = dram.tile([T, D], FP8, tag="rs1i", name="rs1_in")
            rs1_out = dram.tile([TH, D], FP8, tag="rs1o", name="rs1_out")
            rs2_in = dram.tile([T, D], FP8, tag="rs2i", name="rs2_in")
            rs2_out = dram.tile([TH, D], FP8, tag="rs2o", name="rs2_out")
            rsd_in = dram.tile([2, 64], FP8, tag="rsdi", name="rsd_in")
            rsd_out = dram.tile([1, 64], FP8, tag="rsdo", name="rsd_out")

            # ---------------- warmup (HAM) + persistent loads
            junk = p1.tile([P, 512], BF16, tag="junk")
            nc.gpsimd.memset(junk[:], 0.25)
            # pre-load the Exp ACT table + scalar-engine preamble during the
            # input DMA phase so heads1's first real exp starts promptly
            ewarm = p1.tile([P, 8], FP32, tag="ewarm")
            nc.scalar.activation(out=ewarm[:], in_=junk[:, 0:8],
                                 func=mybir.ActivationFunctionType.Exp)
            jp = [psum.tile([P, 512], FP32, tag="st", name=f"jp{i}")
                  for i in range(2)]
            for i in range(24):
                nc.tensor.matmul(jp[i % 2][:], lhsT=junk[:, 0:P], rhs=junk[:],
                                 start=True, stop=True, skip_group_check=True)

            # tiny dummy collective: absorbs the ~11us first-cc stream
            # spin-up so RS1 starts promptly mid-heads2
            zt = p1.tile([2, 64], FP8, tag="zt")
            nc.gpsimd.memset(zt[:], 0.0)
            nc.gpsimd.dma_start(rsd_in[:], zt[:])
            nc.gpsimd.collective_compute(
                "ReduceScatter", mybir.AluOpType.add, replica_groups=groups,
                ins=[rsd_in.opt()], outs=[rsd_out.opt()])

            wq_sb = p1.tile([P, ND, 512], FP8, tag="wq", name="wq1_sb")
            wk_sb = p1.tile([P, ND, 512], FP8, tag="wk", name="wk1_sb")
            nc.sync.dma_start(wq_sb[:], wq1[:])
            nc.sync.dma_start(wk_sb[:], wk1[:])
            yT_sb = p1.tile([P, ND, T], FP8, tag="yT")
            for dc in range(ND):
                nc.sync.dma_start(yT_sb[:, dc, :], yT[:, dc, :])
            S = p1.tile([P, NTO, D], FP32, tag="S")
            for t_ in range(NTO):
                nc.sync.dma_start(S[:, t_, :], ynb[t_ * P:(t_ + 1) * P, :])
            wv_sb = p1.tile([P, ND, 512], FP8, tag="wv", name="wv1_sb")
            nc.sync.dma_start(wv_sb[:], wv1[:])
            xT_sb = p1.tile([P, ND, T], FP8, tag="xT")
            for dc in range(ND):
                nc.sync.dma_start(xT_sb[:, dc, :], xT[:, dc, :])
            yTo_sb = p1.tile([P, ND, TH], BF16, tag="yTo")
            nc.sync.dma_start(yTo_sb[:], yTo[:])
            bi_sb = p1.tile([P, NF], FP32, tag="bi")
            nc.sync.dma_start(bi_sb[:], bi[:])

            # ---------------- building blocks
            def qk_pair(wsb, dst, p, rhs_sb):
                """Project one 128-col block (head pair p) of Q or K via
                DoubleRow fp8. Uses the 2-bank "sc" psum; only OUTSIDE
                head loops."""
                ps = psum.tile([P, T], FP32, tag="sc", name=f"qk_{dst.name}_{p}")
                for dcp in range(ND // 2):
                    for th in range(2):
                        nc.tensor.matmul(
                            ps[:, th * TH:(th + 1) * TH],
                            lhsT=wsb[:, 2 * dcp:2 * dcp + 2, p * P:(p + 1) * P],
                            rhs=rhs_sb[:, 2 * dcp:2 * dcp + 2,
                                       th * TH:(th + 1) * TH],
                            start=(dcp == 0), stop=(dcp == ND // 2 - 1),
                            perf_mode=DR, skip_group_check=True)
                nc.vector.tensor_copy(out=dst[:, p, :], in_=ps[:])

            def qk_half(wsb, dst, p, th, rhs_sb):
                """Stuffable DR half projection using a 1-bank "st" tile."""
                ps = psum.tile([P, TH], FP32, tag="st",
                               name=f"qkh_{dst.name}_{p}_{th}")
                for dcp in range(ND // 2):
                    nc.tensor.matmul(
                        ps[:],
                        lhsT=wsb[:, 2 * dcp:2 * dcp + 2, p * P:(p + 1) * P],
                        rhs=rhs_sb[:, 2 * dcp:2 * dcp + 2,
                                   th * TH:(th + 1) * TH],
                        start=(dcp == 0), stop=(dcp == ND // 2 - 1),
                        perf_mode=DR)
                nc.vector.tensor_copy(out=dst[:, p, th * TH:(th + 1) * TH],
                                      in_=ps[:])

            def v_unit(wva, st, kv_sb):
                pv = psum.tile([P, 512], FP32, tag="st", name=f"v_{wva.name}_{st}")
                for dcp in range(ND // 2):
                    nc.tensor.matmul(
                        pv[:],
                        lhsT=kv_sb[:, 2 * dcp:2 * dcp + 2,
                                   st * P:(st + 1) * P],
                        rhs=wv_sb[:, 2 * dcp:2 * dcp + 2, :],
                        start=(dcp == 0), stop=(dcp == ND // 2 - 1),
                        perf_mode=DR)
                nc.vector.tensor_copy(out=wva[:, st, :], in_=pv[:])

            def h1_unit(h1_sb, fc):
                wi_c = p1.tile([P, ND, P], BF16, tag="wic", bufs=3,
                               name=f"wic_{fc}")
                nc.sync.dma_start(wi_c[:], wi[fc])
                ph = psum.tile([P, TH], FP32, tag="st", name=f"h1_{fc}")
                for dc in range(ND):
                    nc.tensor.matmul(
                        ph[:], lhsT=wi_c[:, dc, :], rhs=yTo_sb[:, dc, :],
                        start=(dc == 0), stop=(dc == ND - 1))
                if fc >= 24:
                    # late units land at the heads2 tail where vector is the
                    # serial bottleneck and scalar has gone idle
                    nc.scalar.activation(
                        out=h1_sb[:, fc, :], in_=ph[:],
                        func=mybir.ActivationFunctionType.Relu,
                        bias=bi_sb[:, fc:fc + 1])
                else:
                    nc.vector.tensor_scalar(
                        out=h1_sb[:, fc, :], in0=ph[:],
                        scalar1=bi_sb[:, fc:fc + 1], scalar2=0.0,
                        op0=mybir.AluOpType.add, op1=mybir.AluOpType.max)

            def wo_unit(pt, wof, tt8, dh, dst_ap, ptag="st", cast_eng="v"):
                """m-partial rows tt8 (full T), d-half dh -> fp8 -> DRAM rs
                buf. DoubleRow over my 512 Wo rows (2 chunk-pairs)."""
                ps = psum.tile([P, TH], FP32, tag=ptag,
                               name=f"wo_{wof.name}_{tt8}_{dh}")
                for cp in range(2):
                    nc.tensor.matmul(
                        ps[:],
                        lhsT=pt[:, 2 * cp:2 * cp + 2, tt8 * P:(tt8 + 1) * P],
                        rhs=wof[:, 2 * cp:2 * cp + 2, dh * TH:(dh + 1) * TH],
                        start=(cp == 0), stop=(cp == 1),
                        perf_mode=DR)
                stg = p1.tile([P, TH], FP8, tag="mstg", bufs=3,
                              name=f"mstg_{wof.name}_{tt8}_{dh}")
                if cast_eng == "s":
                    # scalar engine is idle post-heads2; split the casts so
                    # the wo2 chain is not vector-serialized
                    nc.scalar.activation(
                        out=stg[:], in_=ps[:],
                        func=mybir.ActivationFunctionType.Copy,
                        scale=RS_SCALE / M_PSUM)
                else:
                    nc.vector.tensor_scalar(
                        out=stg[:], in0=ps[:], scalar1=RS_SCALE / M_PSUM,
                        scalar2=0.0, op0=mybir.AluOpType.mult,
                        op1=mybir.AluOpType.bypass)
                nc.sync.dma_start(dst_ap, stg[:])

            def m_add(rs_out_ap, tt, tag):
                """S[tt] += rs_out / RS_SCALE (one own-row tile). Load AND
                add both ride gpsimd (idle), keeping the sync DMA stream and
                the vector engine free of collective-gated work."""
                mld = p1.tile([P, D], FP8, tag="mld", bufs=4,
                              name=f"mld_{tag}_{tt}")
                nc.gpsimd.dma_start(mld[:], rs_out_ap)
                nc.vector.scalar_tensor_tensor(
                    out=S[:, tt, :], in0=mld[:], scalar=1.0 / RS_SCALE,
                    in1=S[:, tt, :],
                    op0=mybir.AluOpType.mult, op1=mybir.AluOpType.add)

            # stuffing queue machinery
            stuff_q = []

            def maybe_stuff(n=1):
                for _ in range(n):
                    if stuff_q:
                        stuff_q.pop(0)()

            def heads(m, wqt, wkt, wva, pt, final_stuff=True):
                """4 head-pairs; scores row-group paired; exp fp8; partial.
                final_stuff=False skips the post-loop stuffing so the NEXT
                heads phase's first scores aren't queued behind ~10us of
                stuffed h1 units (a 15us scalar hole at the boundary)."""
                prev = None

                def partial_block(p, exA, exB, den):
                    rden = small.tile([P, 2, NT], FP32, tag="rden",
                                      name=f"rden{m}_{p}")
                    nc.vector.reciprocal(out=rden[:], in_=den[:])
                    wvp = small.tile([P, 2, NT, DV], FP8, tag="wvp",
                                     name=f"wvp{m}_{p}")
                    for j in range(2):
                        for st in range(NT):
                            nc.vector.tensor_scalar(
                                out=wvp[:, j, st, :],
                                in0=wva[:, st, (2 * p + j) * DV:(2 * p + j + 1) * DV],
                                scalar1=rden[:, j, st:st + 1],
                                scalar2=WVP_BOOST,
                                op0=mybir.AluOpType.mult,
                                op1=mybir.AluOpType.mult)
                    pps = [psum.tile([P, TH], FP32, tag="pp", name=f"pp{m}_{p}_{j}")
                           for j in range(2)]
                    for j, ex in ((0, exA), (1, exB)):
                        for st in range(NT):
                            nc.tensor.matmul(
                                pps[j][0:64, :], lhsT=wvp[:, j, st, :],
                                rhs=ex[st][:, 0:TH],
                                start=(st == 0), stop=(st == NT - 1),
                                skip_group_check=True)
                            nc.tensor.matmul(
                                pps[j][64:128, :], lhsT=wvp[:, j, st, :],
                                rhs=ex[st][:, TH:T],
                                start=(st == 0), stop=(st == NT - 1),
                                tile_position=(0, 64), skip_group_check=True)
                    for j in range(2):
                        lo, hi = 64 * j, 64 * j + 64
                        nc.vector.tensor_copy(out=pt[lo:hi, p, 0:TH],
                                              in_=pps[j][0:64, :])
                        nc.vector.tensor_copy(out=pt[lo:hi, p, TH:T],
                                              in_=pps[j][64:128, :])

                for p in range(4):
                    den = small.tile([P, 2, NT], FP32, tag="den",
                                     name=f"den{m}_{p}")
                    exA, exB = [], []
                    for st in range(NT):
                        psA = psum.tile([P, T], FP32, tag="sc",
                                        name=f"scA{m}_{p}_{st}")
                        psB = psum.tile([P, T], FP32, tag="sc",
                                        name=f"scB{m}_{p}_{st}")
                        for th in range(2):
                            tsl = slice(th * TH, (th + 1) * TH)
                            nc.tensor.matmul(
                                psA[:, tsl],
                                lhsT=wkt[0:64, p, st * P:(st + 1) * P],
                                rhs=wqt[0:64, p, tsl],
                                start=True, stop=True, skip_group_check=True)
                            nc.tensor.matmul(
                                psB[:, tsl],
                                lhsT=wkt[64:128, p, st * P:(st + 1) * P],
                                rhs=wqt[64:128, p, tsl],
                                start=True, stop=True, skip_group_check=True)
                        eA = expp.tile([P, T], FP8, tag="exp",
                                       name=f"exA{m}_{p}_{st}")
                        nc.scalar.activation(
                            out=eA[:], in_=psA[:],
                            func=mybir.ActivationFunctionType.Exp,
                            scale=1.0 / (QK_SCALE * QK_SCALE),
                            accum_out=den[:, 0, st:st + 1])
                        eB = expp.tile([P, T], FP8, tag="exp",
                                       name=f"exB{m}_{p}_{st}")
                        nc.scalar.activation(
                            out=eB[:], in_=psB[:],
                            func=mybir.ActivationFunctionType.Exp,
                            scale=1.0 / (QK_SCALE * QK_SCALE),
                            accum_out=den[:, 1, st:st + 1])
                        exA.append(eA)
                        exB.append(eB)
                        if st < 7:
                            maybe_stuff(1)
                    if prev is not None:
                        partial_block(*prev)
                        maybe_stuff(3)
                    prev = (p, exA, exB, den)
                partial_block(*prev)
                if final_stuff:
                    maybe_stuff(3)

            # ---------------- QKV1 projections (PE dense from the start)
            wqt1 = p1.tile([P, 4, T], FP8, tag="wqt", bufs=2, name="wqt1")
            wkt1 = p1.tile([P, 4, T], FP8, tag="wkt", bufs=2, name="wkt1")
            for p in range(4):
                qk_pair(wq_sb, wqt1, p, yT_sb)
                qk_pair(wk_sb, wkt1, p, yT_sb)

            wva1 = p1.tile([P, NT, 512], FP8, tag="wva", bufs=2, name="wva1")
            wva2 = p1.tile([P, NT, 512], FP8, tag="wva", bufs=2, name="wva2")
            h1_sb = p1.tile([P, NF, TH], BF16, tag="h1")

            wq2_sb = p1.tile([P, ND, 512], FP8, tag="wq", name="wq2_sb")
            wk2_sb = p1.tile([P, ND, 512], FP8, tag="wk", name="wk2_sb")
            wv2_sb = p1.tile([P, ND, 512], FP8, tag="wv", name="wv2_sb")
            wqt2 = p1.tile([P, 4, T], FP8, tag="wqt", bufs=2, name="wqt2")
            wkt2 = p1.tile([P, 4, T], FP8, tag="wkt", bufs=2, name="wkt2")

            def load_w2():
                nc.sync.dma_start(wq2_sb[:], wq2[:])
                nc.sync.dma_start(wk2_sb[:], wk2[:])

            def load_wv2():
                nc.sync.dma_start(wv2_sb[:], wv2[:])

            def qk2_units(p):
                return [lambda th=th: qk_half(wq2_sb, wqt2, p, th, yT_sb)
                        for th in range(2)] + \
                       [lambda th=th: qk_half(wk2_sb, wkt2, p, th, xT_sb)
                        for th in range(2)]

            # heads1 stuffing: v1 / v2 / qk2 p0-p3 / h1 (tail spills to heads2)
            for st in range(NT):
                stuff_q.append(lambda st=st: v_unit(wva1, st, yT_sb))
            stuff_q.append(load_w2)
            stuff_q.append(load_wv2)
            stuff_q.extend(qk2_units(0))
            for st in range(NT):
                stuff_q.append(lambda st=st: v_unit(wva2, st, xT_sb))
            stuff_q.extend(qk2_units(1))
            for fc in range(8):
                stuff_q.append(lambda fc=fc: h1_unit(h1_sb, fc))
            stuff_q.extend(qk2_units(2))
            stuff_q.extend(qk2_units(3))
            for fc in range(8, 16):
                stuff_q.append(lambda fc=fc: h1_unit(h1_sb, fc))

            # ---------------- heads1 (scalar-bound; stuffed)
            pt1 = p1.tile([P, 4, T], FP8, tag="pt", bufs=2, name="pt1")
            heads(1, wqt1, wkt1, wva1, pt1, final_stuff=False)

            # ---------------- m1 = pt1 @ Wo1 (full T) -> RS1. The wo1 units
            # go FIRST in the heads2 stuff queue (cheap with DR), so RS1
            # triggers ~15us into heads2; the m1 adds are the LAST stuffed
            # closures, giving the collective the whole phase to land.
            wo1f = p1.tile([P, 4, D], FP8, tag="wof", bufs=2, name="wo1f")
            nc.sync.dma_start(wo1f[:], wo1[:])
            wo2f = p1.tile([P, 4, D], FP8, tag="wof", bufs=2, name="wo2f")
            nc.sync.dma_start(wo2f[:], wo2[:])

            for tt8 in range(NT):
                for dh in range(2):
                    stuff_q.append(lambda tt8=tt8, dh=dh: wo_unit(
                        pt1, wo1f, tt8, dh,
                        rs1_in[tt8 * P:(tt8 + 1) * P, dh * TH:(dh + 1) * TH]))

            def rs1_cc():
                nc.gpsimd.collective_compute(
                    "ReduceScatter", mybir.AluOpType.add, replica_groups=groups,
                    ins=[rs1_in.opt()], outs=[rs1_out.opt()])

            stuff_q.append(rs1_cc)
            for fc in range(16, NF):
                stuff_q.append(lambda fc=fc: h1_unit(h1_sb, fc))

            # ---------------- heads2
            pt2 = p1.tile([P, 4, T], FP8, tag="pt", bufs=2, name="pt2")
            heads(2, wqt2, wkt2, wva2, pt2)
            while stuff_q:
                maybe_stuff(1)

            # ---------------- m2 -> RS2 (hidden under the first ffp sweep).
            # Prefetch the first wot chunks so the sweep starts without
            # waiting behind the wo2 stg DMAs on the sync queue.
            wotc_pre = []
            for fc in range(4):
                wc = p1.tile([P, D], BF16, tag="wotc", bufs=4,
                             name=f"wotc_{fc}")
                nc.sync.dma_start(wc[:], wot[fc])
                wotc_pre.append(wc)
            for k, (tt8, dh) in enumerate((t8, d) for t8 in range(NT)
                                          for d in range(2)):
                wo_unit(pt2, wo2f, tt8, dh,
                        rs2_in[tt8 * P:(tt8 + 1) * P,
                               dh * TH:(dh + 1) * TH],
                        ptag=("st", "pp")[k % 2],
                        cast_eng=("v", "s")[k % 2])
            nc.gpsimd.collective_compute(
                "ReduceScatter", mybir.AluOpType.add, replica_groups=groups,
                ins=[rs2_in.opt()], outs=[rs2_out.opt()])

            def final_chain(tt):
                stats = small.tile([P, 2, 6], FP32, tag="stats",
                                   name=f"stats_{tt}")
                for i in range(2):
                    nc.vector.bn_stats(out=stats[:, i, :],
                                       in_=S[:, tt, i * TH:(i + 1) * TH])
                mv = small.tile([P, 2], FP32, tag="mv", name=f"mv_{tt}")
                nc.vector.bn_aggr(out=mv[:], in_=stats[:])
                std = small.tile([P, 1], FP32, tag="std", name=f"std_{tt}")
                nc.scalar.activation(
                    out=std[:], in_=mv[:, 1:2],
                    func=mybir.ActivationFunctionType.Sqrt,
                    scale=float(D) / float(D - 1))
                msum = small.tile([P, 1], FP32, tag="msum", name=f"msum_{tt}")
                nc.vector.tensor_add(out=msum[:], in0=mv[:, 0:1], in1=std[:])
                nc.vector.tensor_scalar_sub(out=S[:, tt, :], in0=S[:, tt, :],
                                            scalar1=msum[:])
                nc.sync.dma_start(out[tt * P:(tt + 1) * P, :], S[:, tt, :])

            # ---------------- endgame: the ffp sweeps hide RS2; each tt's
            # m1/m2 adds ride after its sweep's S-adds (m1 adds any earlier
            # head-of-line-block the vector queue on RS1 and starve wo2).
            def ffp_sweep(tts):
                acc = [psum.tile([P, T], FP32, tag="sc", name=f"ffa_{tts[0]}_{k}")
                       for k in range(2)]
                for fc in range(NF):
                    if tts[0] == 0 and fc < 4:
                        wot_c = wotc_pre[fc]
                    else:
                        wot_c = p1.tile([P, D], BF16, tag="wotc", bufs=4,
                                        name=f"wotc_{tts[0]}_{fc}")
                        nc.sync.dma_start(wot_c[:], wot[fc])
                    for k, tt in enumerate(tts):
                        for dh in range(2):
                            nc.tensor.matmul(
                                acc[k][:, dh * TH:(dh + 1) * TH],
                                lhsT=h1_sb[:, fc, tt * P:(tt + 1) * P],
                                rhs=wot_c[:, dh * TH:(dh + 1) * TH],
                                start=(fc == 0), stop=(fc == NF - 1),
                                skip_group_check=True)
                for k, tt in enumerate(tts):
                    for dh in range(2):
                        nc.vector.tensor_add(
                            out=S[:, tt, dh * TH:(dh + 1) * TH],
                            in0=acc[k][:, dh * TH:(dh + 1) * TH],
                            in1=S[:, tt, dh * TH:(dh + 1) * TH])

            ffp_sweep((0, 1))
            m_add(rs1_out[0 * P:1 * P, :], 0, "m1")
            m_add(rs1_out[1 * P:2 * P, :], 1, "m1")
            m_add(rs2_out[0 * P:1 * P, :], 0, "m2")
            m_add(rs2_out[1 * P:2 * P, :], 1, "m2")
            final_chain(0)
            final_chain(1)
            # tt2/3's m adds ride under sweep B (RS1/RS2 land before or
            # early in it), leaving only S-adds + chains as the tail
            m_add(rs1_out[2 * P:3 * P, :], 2, "m1")
            m_add(rs1_out[3 * P:4 * P, :], 3, "m1")
            m_add(rs2_out[2 * P:3 * P, :], 2, "m2")
            m_add(rs2_out[3 * P:4 * P, :], 3, "m2")
            ffp_sweep((2, 3))
            final_chain(2)
            final_chain(3)

    if compile:
        nc.compile()
    return nc


# ---------------------------------------------------------------- host side

def pack_inputs(x, y, Wq1, Wk1, Wv1, Wo1, Wq2, Wk2, Wv2, Wo2,
                W_in, b_in, W_out, b_out):
    NH = H // 2
    NPFP8 = mybir.dt.np(FP8)

    def q8(a):
        return np.clip(a, -240.0, 240.0).astype(NPFP8)

    def tr8(a):                # [T, D] -> fp8 [128, ND, T]
        return q8(np.ascontiguousarray(
            a.T.reshape(ND, P, T).transpose(1, 0, 2)))

    def trb(a):                # [T, D] -> bf16 [128, ND, T-slice]
        return np.ascontiguousarray(
            a.T.reshape(ND, P, a.shape[0]).transpose(1, 0, 2)).astype(NPBF16)

    def qk_pack(W, h0):        # [H,D,DK] -> fp8 [128, ND, 512] pair-blocked
        Wh = W[h0:h0 + NH]
        Wp = Wh.reshape(NH // 2, 2, D, DK).transpose(2, 0, 1, 3)
        Wp = Wp.reshape(D, NH * DK)
        return q8(np.ascontiguousarray(
            Wp.reshape(ND, P, NH * DK).transpose(1, 0, 2)))

    def v_pack(W, h0):
        Wh = W[h0:h0 + NH].transpose(1, 0, 2).reshape(D, NH * DV)
        return q8(np.ascontiguousarray(
            Wh.reshape(ND, P, NH * DV).transpose(1, 0, 2)))

    def wo_pack(Wo, h):        # my half rows of Wo -> fp8 [128, 4, D]
        Ws = Wo[NH * DV * h:NH * DV * (h + 1)] * WO_SCALE
        return q8(np.ascontiguousarray(
            Ws.reshape(4, P, D).transpose(1, 0, 2)))

    def wi_pack(W):            # [FF, D] -> bf16 [NF, 128, ND, 128]
        A = W.T.reshape(ND, P, NF, P)
        return np.ascontiguousarray(A.transpose(2, 1, 0, 3)).astype(NPBF16)

    def wot_pack(W):           # [D, FF] -> bf16 [NF, 128, D]
        return np.ascontiguousarray(
            W.T.reshape(NF, P, D)).astype(NPBF16)

    # scores need /sqrt(DK) total; QK_SCALE boost per side is undone by the
    # exp scale (1/QK_SCALE^2).
    sq = np.float32(QK_SCALE / np.sqrt(np.sqrt(np.float32(DK))))

    wi_p = wi_pack(np.asarray(W_in))
    wot_p = wot_pack(np.asarray(W_out))
    bi_p = np.ascontiguousarray(
        np.asarray(b_in).reshape(NF, P).T).astype(np.float32)

    in_maps = []
    for c in range(2 * x.shape[0]):
        b, h = c // 2, c % 2
        h0 = NH * h
        yb, xb = np.asarray(y[b]), np.asarray(x[b])
        in_maps.append(dict(
            yT=tr8(yb),
            xT=tr8(xb),
            yTo=trb(yb)[:, :, h * TH:(h + 1) * TH].copy(),
            ynb=(yb[h * TH:(h + 1) * TH] +
                 np.asarray(b_out)[None, :]).astype(np.float32),
            wq1=qk_pack(Wq1 * sq, h0), wk1=qk_pack(Wk1 * sq, h0),
            wv1=v_pack(Wv1 * WV_SCALE, h0), wo1=wo_pack(np.asarray(Wo1), h),
            wq2=qk_pack(Wq2 * sq, h0), wk2=qk_pack(Wk2 * sq, h0),
            wv2=v_pack(Wv2 * WV_SCALE, h0), wo2=wo_pack(np.asarray(Wo2), h),
            wi=wi_p, wot=wot_p, bi=bi_p,
        ))
    return in_maps


_PROG_CACHE = {}


def kernel(**inputs) -> np.ndarray:
    inputs = {k: np.asarray(v, np.float32) for k, v in inputs.items()}
    if "full" not in _PROG_CACHE:
        _PROG_CACHE["full"] = build_program()
    nc = _PROG_CACHE["full"]
    in_maps = pack_inputs(**inputs)
    res = run_bass_kernel_spmd(nc, in_maps, core_ids=list(range(8)))
    out = np.empty((B, T, D), np.float32)
    for c in range(8):
        b, h = c // 2, c % 2
        out[b, h * TH:(h + 1) * TH] = res.results[c]["out"]
    return out


# revision 77
# speedup vs baseline: 1.2739x; 1.0018x over previous
"""Trainium2 Bass kernel for nn_DecoderStack — v4.

Changes vs v3 (469us):
  * fp8 DoubleRow matmuls (2x PE throughput, measured on hw) for the QKV
    projections and the Wo applications. Weights are rescaled into fp8's
    normal range (Wq/Wk x16*sq with exp(scale=1/256); Wo x4; V x48) so
    quantization stays ~6%/element instead of drowning in subnormals.
  * The two ReduceScatters stay (pair-rank routing needs a sum collective;
    AllToAll is mesh-only <4 cores) but are rescheduled: wo1 units are ~3x
    cheaper (DoubleRow), stuffed first into heads2, so RS1 flies fully
    under heads2 (scalar-bound there, so the HAM CC-throttle is cheap);
    a tiny dummy RS at t=0 absorbs the ~11us first-cc stream spin-up.
    m1 adds moved to the endgame (their RS-gated vector STTs stalled the
    vector queue inside heads2 otherwise). RS2 hides under ffp sweep A.
  * FFN stays bf16 (fp8 there measurably blows the 2e-2 error budget).
  * pt1/pt2 distinct buffers; wo2 psum rotates st+pp tags (4 banks) and
    casts alternate vector/scalar so the post-heads2 chain to the RS2
    trigger is not vector-serialized; h1 relu for fc>=24 runs on scalar.
  * Measured 360-367us typical, best 360.2 (from 469us baseline); rel
    err 1.354e-2. Occasional runs land ~395-400 when the pair
    collectives run slow (inter-core skew / neighbor drift) - treat
    single-run deltas <15us as noise.
    Final wins: heads1 skips post-loop stuffing (final_stuff=False) so
    heads2's first scores aren't behind ~10us of h1 units; junk warmup
    24; ALL m-adds ride after/under the ffp sweeps (an m-add issued
    while its RS is in flight head-of-line-blocks the vector queue and
    starves wo2's psum rotation); wq/wk DMAs before yT chunks; tt2/3
    m-adds hoisted before sweep B.
    Known remaining costs: ~12us PE half-clock (HAM) while RS2 flies
    under ffp sweep A; ~180us scalar exp chain is the heads floor; ~8us
    framework preamble; ~30us ramp to first exp. Failed experiments
    (scheduler reorders defeat intuition - A/B everything, and re-run
    regressions before reverting): early-heads QKV1 split, cost-aware
    stuff budgeting, all-scalar wo2 casts, junk < 20, single-DMA
    yT/xT, wotc bufs=8 + m1(0,1) before sweep A.
"""

import sys

for _p in ("/opt/trn_rl_repo", "/root/.axon_site"):
    if _p not in sys.path:
        sys.path.insert(0, _p)

import contextlib

import numpy as np

import concourse.bass as bass
import concourse.bacc as bacc
import concourse.tile as tile
from concourse import mybir
from concourse.bass_utils import run_bass_kernel_spmd

B, T, D, H, DK, DV, FF = 4, 1024, 1024, 16, 64, 64, 4096
P = 128
TH = T // 2           # rows owned per core
NT = T // P           # 8 t/s tiles over full T
ND = D // P           # 8 d chunks
NF = FF // P          # 32 ff chunks
NTO = TH // P         # 4 own-row tiles
FP32 = mybir.dt.float32
BF16 = mybir.dt.bfloat16
FP8 = mybir.dt.float8e4
DR = mybir.MatmulPerfMode.DoubleRow
NPBF16 = mybir.dt.np(BF16)
QK_SCALE = 16.0       # per-side boost on Wq/Wk packs (scores x256 -> exp scale)
WV_SCALE = 48.0       # fp8: wv*48 keeps |V| < 240
WVP_BOOST = 8.0       # extra boost on wvp (= wva/denom) to avoid denormals
WO_SCALE = 4.0        # Wo pack boost into fp8 normal range
RS_SCALE = 16.0       # m-branch RS payload in fp8 at 16x scale
M_PSUM = WV_SCALE * WVP_BOOST * WO_SCALE   # wo psum = M_PSUM * m


def build_program(n_cores: int = 8, compile: bool = True):
    nc = bacc.Bacc("TRN2", target_bir_lowering=False, debug=False,
                   num_devices=n_cores)
    groups = [[2 * g, 2 * g + 1] for g in range(n_cores // 2)]

    def dram_in(name, shape, dt=FP8):
        return nc.dram_tensor(name, shape, dt, kind="ExternalInput")

    yT = dram_in("yT", [P, ND, T])
    xT = dram_in("xT", [P, ND, T])
    yTo = dram_in("yTo", [P, ND, TH], BF16)
    ynb = dram_in("ynb", [TH, D], FP32)        # y own rows + b_out
    wq1 = dram_in("wq1", [P, ND, 512])
    wk1 = dram_in("wk1", [P, ND, 512])
    wv1 = dram_in("wv1", [P, ND, 512])
    wo1 = dram_in("wo1", [P, 4, D])
    wq2 = dram_in("wq2", [P, ND, 512])
    wk2 = dram_in("wk2", [P, ND, 512])
    wv2 = dram_in("wv2", [P, ND, 512])
    wo2 = dram_in("wo2", [P, 4, D])
    wi = dram_in("wi", [NF, P, ND, P], BF16)
    wot = dram_in("wot", [NF, P, D], BF16)
    bi = dram_in("bi", [P, NF], FP32)
    out = nc.dram_tensor("out", [TH, D], FP32, kind="ExternalOutput")

    with tile.TileContext(nc) as tc:
        with contextlib.ExitStack() as ctx:
            p1 = ctx.enter_context(tc.tile_pool(name="p1", bufs=1))
            expp = ctx.enter_context(tc.tile_pool(name="expp", bufs=20))
            small = ctx.enter_context(tc.tile_pool(name="small", bufs=2))
            psum = ctx.enter_context(tc.tile_pool(name="psum", bufs=2, space="PSUM"))
            dram = ctx.enter_context(tc.tile_pool(name="dram", bufs=1, space="DRAM"))

            rs1_in = dram.tile([T, D], FP8, tag="rs1i", name="rs1_in")
            rs1_out = dram.tile([TH, D], FP8, tag="rs1o", name="rs1_out")
            rs2_in = dram.tile([T, D], FP8, tag="rs2i", name="rs2_in")
            rs2_out = dram.tile([TH, D], FP8, tag="rs2o", name="rs2_out")
            rsd_in = dram.tile([2, 64], FP8, tag="rsdi", name="rsd_in")
            rsd_out = dram.tile([1, 64], FP8, tag="rsdo", name="rsd_out")

            # ---------------- warmup (HAM) + persistent loads
            junk = p1.tile([P, 512], BF16, tag="junk")
            nc.gpsimd.memset(junk[:], 0.25)
            # pre-load the Exp ACT table + scalar-engine preamble during the
            # input DMA phase so heads1's first real exp starts promptly
            ewarm = p1.tile([P, 8], FP32, tag="ewarm")
            nc.scalar.activation(out=ewarm[:], in_=junk[:, 0:8],
                                 func=mybir.ActivationFunctionType.Exp)
            jp = [psum.tile([P, 512], FP32, tag="st", name=f"jp{i}")
                  for i in range(2)]
            for i in range(24):
                nc.tensor.matmul(jp[i % 2][:], lhsT=junk[:, 0:P], rhs=junk[:],
                                 start=True, stop=True, skip_group_check=True)

            # tiny dummy collective: absorbs the ~11us first-cc stream
            # spin-up so RS1 starts promptly mid-heads2
            zt = p1.tile([2, 64], FP8, tag="zt")
            nc.gpsimd.memset(zt[:], 0.0)
            nc.gpsimd.dma_start(rsd_in[:], zt[:])
            nc.gpsimd.collective_compute(
                "ReduceScatter", mybir.AluOpType.add, replica_groups=groups,
                ins=[rsd_in.opt()], outs=[rsd_out.opt()])

            wq_sb = p1.tile([P, ND, 512], FP8, tag="wq", name="wq1_sb")
            wk_sb = p1.tile([P, ND, 512], FP8, tag="wk", name="wk1_sb")
            nc.sync.dma_start(wq_sb[:], wq1[:])
            nc.sync.dma_start(wk_sb[:], wk1[:])
            yT_sb = p1.tile([P, ND, T], FP8, tag="yT")
            for dc in range(ND):
                nc.sync.dma_start(yT_sb[:, dc, :], yT[:, dc, :])
            S = p1.tile([P, NTO, D], FP32, tag="S")
            for t_ in range(NTO):
                nc.sync.dma_start(S[:, t_, :], ynb[t_ * P:(t_ + 1) * P, :])
            wv_sb = p1.tile([P, ND, 512], FP8, tag="wv", name="wv1_sb")
            nc.sync.dma_start(wv_sb[:], wv1[:])
            xT_sb = p1.tile([P, ND, T], FP8, tag="xT")
            for dc in range(ND):
                nc.sync.dma_start(xT_sb[:, dc, :], xT[:, dc, :])
            yTo_sb = p1.tile([P, ND, TH], BF16, tag="yTo")
            nc.sync.dma_start(yTo_sb[:], yTo[:])
            bi_sb = p1.tile([P, NF], FP32, tag="bi")
            nc.sync.dma_start(bi_sb[:], bi[:])

            # ---------------- building blocks
            def qk_pair(wsb, dst, p, rhs_sb):
                """Project one 128-col block (head pair p) of Q or K via
                DoubleRow fp8. Uses the 2-bank "sc" psum; only OUTSIDE
                head loops."""
                ps = psum.tile([P, T], FP32, tag="sc", name=f"qk_{dst.name}_{p}")
                for dcp in range(ND // 2):
                    for th in range(2):
                        nc.tensor.matmul(
                            ps[:, th * TH:(th + 1) * TH],
                            lhsT=wsb[:, 2 * dcp:2 * dcp + 2, p * P:(p + 1) * P],
                            rhs=rhs_sb[:, 2 * dcp:2 * dcp + 2,
                                       th * TH:(th + 1) * TH],
                            start=(dcp == 0), stop=(dcp == ND // 2 - 1),
                            perf_mode=DR, skip_group_check=True)
                nc.vector.tensor_copy(out=dst[:, p, :], in_=ps[:])

            def qk_half(wsb, dst, p, th, rhs_sb):
                """Stuffable DR half projection using a 1-bank "st" tile."""
                ps = psum.tile([P, TH], FP32, tag="st",
                               name=f"qkh_{dst.name}_{p}_{th}")
                for dcp in range(ND // 2):
                    nc.tensor.matmul(
                        ps[:],
                        lhsT=wsb[:, 2 * dcp:2 * dcp + 2, p * P:(p + 1) * P],
                        rhs=rhs_sb[:, 2 * dcp:2 * dcp + 2,
                                   th * TH:(th + 1) * TH],
                        start=(dcp == 0), stop=(dcp == ND // 2 - 1),
                        perf_mode=DR)
                nc.vector.tensor_copy(out=dst[:, p, th * TH:(th + 1) * TH],
                                      in_=ps[:])

            def v_unit(wva, st, kv_sb):
                pv = psum.tile([P, 512], FP32, tag="st", name=f"v_{wva.name}_{st}")
                for dcp in range(ND // 2):
                    nc.tensor.matmul(
                        pv[:],
                        lhsT=kv_sb[:, 2 * dcp:2 * dcp + 2,
                                   st * P:(st + 1) * P],
                        rhs=wv_sb[:, 2 * dcp:2 * dcp + 2, :],
                        start=(dcp == 0), stop=(dcp == ND // 2 - 1),
                        perf_mode=DR)
                nc.vector.tensor_copy(out=wva[:, st, :], in_=pv[:])

            def h1_unit(h1_sb, fc):
                wi_c = p1.tile([P, ND, P], BF16, tag="wic", bufs=3,
                               name=f"wic_{fc}")
                nc.sync.dma_start(wi_c[:], wi[fc])
                ph = psum.tile([P, TH], FP32, tag="st", name=f"h1_{fc}")
                for dc in range(ND):
                    nc.tensor.matmul(
                        ph[:], lhsT=wi_c[:, dc, :], rhs=yTo_sb[:, dc, :],
                        start=(dc == 0), stop=(dc == ND - 1))
                if fc >= 24:
                    # late units land at the heads2 tail where vector is the
                    # serial bottleneck and scalar has gone idle
                    nc.scalar.activation(
                        out=h1_sb[:, fc, :], in_=ph[:],
                        func=mybir.ActivationFunctionType.Relu,
                        bias=bi_sb[:, fc:fc + 1])
                else:
                    nc.vector.tensor_scalar(
                        out=h1_sb[:, fc, :], in0=ph[:],
                        scalar1=bi_sb[:, fc:fc + 1], scalar2=0.0,
                        op0=mybir.AluOpType.add, op1=mybir.AluOpType.max)

            def wo_unit(pt, wof, tt8, dh, dst_ap, ptag="st", cast_eng="v"):
                """m-partial rows tt8 (full T), d-half dh -> fp8 -> DRAM rs
                buf. DoubleRow over my 512 Wo rows (2 chunk-pairs)."""
                ps = psum.tile([P, TH], FP32, tag=ptag,
                               name=f"wo_{wof.name}_{tt8}_{dh}")
                for cp in range(2):
                    nc.tensor.matmul(
                        ps[:],
                        lhsT=pt[:, 2 * cp:2 * cp + 2, tt8 * P:(tt8 + 1) * P],
                        rhs=wof[:, 2 * cp:2 * cp + 2, dh * TH:(dh + 1) * TH],
                        start=(cp == 0), stop=(cp == 1),
                        perf_mode=DR)
                stg = p1.tile([P, TH], FP8, tag="mstg", bufs=3,
                              name=f"mstg_{wof.name}_{tt8}_{dh}")
                if cast_eng == "s":
                    # scalar engine is idle post-heads2; split the casts so
                    # the wo2 chain is not vector-serialized
                    nc.scalar.activation(
                        out=stg[:], in_=ps[:],
                        func=mybir.ActivationFunctionType.Copy,
                        scale=RS_SCALE / M_PSUM)
                else:
                    nc.vector.tensor_scalar(
                        out=stg[:], in0=ps[:], scalar1=RS_SCALE / M_PSUM,
                        scalar2=0.0, op0=mybir.AluOpType.mult,
                        op1=mybir.AluOpType.bypass)
                nc.sync.dma_start(dst_ap, stg[:])

            def m_add(rs_out_ap, tt, tag):
                """S[tt] += rs_out / RS_SCALE (one own-row tile). Load AND
                add both ride gpsimd (idle), keeping the sync DMA stream and
                the vector engine free of collective-gated work."""
                mld = p1.tile([P, D], FP8, tag="mld", bufs=4,
                              name=f"mld_{tag}_{tt}")
                nc.gpsimd.dma_start(mld[:], rs_out_ap)
                nc.vector.scalar_tensor_tensor(
                    out=S[:, tt, :], in0=mld[:], scalar=1.0 / RS_SCALE,
                    in1=S[:, tt, :],
                    op0=mybir.AluOpType.mult, op1=mybir.AluOpType.add)

            # stuffing queue machinery
            stuff_q = []

            def maybe_stuff(n=1):
                for _ in range(n):
                    if stuff_q:
                        stuff_q.pop(0)()

            def heads(m, wqt, wkt, wva, pt, final_stuff=True):
                """4 head-pairs; scores row-group paired; exp fp8; partial.
                final_stuff=False skips the post-loop stuffing so the NEXT
                heads phase's first scores aren't queued behind ~10us of
                stuffed h1 units (a 15us scalar hole at the boundary)."""
                prev = None

                def partial_block(p, exA, exB, den):
                    rden = small.tile([P, 2, NT], FP32, tag="rden",
                                      name=f"rden{m}_{p}")
                    nc.vector.reciprocal(out=rden[:], in_=den[:])
                    wvp = small.tile([P, 2, NT, DV], FP8, tag="wvp",
                                     name=f"wvp{m}_{p}")
                    for j in range(2):
                        for st in range(NT):
                            nc.vector.tensor_scalar(
                                out=wvp[:, j, st, :],
                                in0=wva[:, st, (2 * p + j) * DV:(2 * p + j + 1) * DV],
                                scalar1=rden[:, j, st:st + 1],
                                scalar2=WVP_BOOST,
                                op0=mybir.AluOpType.mult,
                                op1=mybir.AluOpType.mult)
                    pps = [psum.tile([P, TH], FP32, tag="pp", name=f"pp{m}_{p}_{j}")
                           for j in range(2)]
                    for j, ex in ((0, exA), (1, exB)):
                        for st in range(NT):
                            nc.tensor.matmul(
                                pps[j][0:64, :], lhsT=wvp[:, j, st, :],
                                rhs=ex[st][:, 0:TH],
                                start=(st == 0), stop=(st == NT - 1),
                                skip_group_check=True)
                            nc.tensor.matmul(
                                pps[j][64:128, :], lhsT=wvp[:, j, st, :],
                                rhs=ex[st][:, TH:T],
                                start=(st == 0), stop=(st == NT - 1),
                                tile_position=(0, 64), skip_group_check=True)
                    for j in range(2):
                        lo, hi = 64 * j, 64 * j + 64
                        nc.vector.tensor_copy(out=pt[lo:hi, p, 0:TH],
                                              in_=pps[j][0:64, :])
                        nc.vector.tensor_copy(out=pt[lo:hi, p, TH:T],
                                              in_=pps[j][64:128, :])

                for p in range(4):
                    den = small.tile([P, 2, NT], FP32, tag="den",
                                     name=f"den{m}_{p}")
                    exA, exB = [], []
                    for st in range(NT):
                        psA = psum.tile([P, T], FP32, tag="sc",
                                        name=f"scA{m}_{p}_{st}")
                        psB = psum.tile([P, T], FP32, tag="sc",
                                        name=f"scB{m}_{p}_{st}")
                        for th in range(2):
                            tsl = slice(th * TH, (th + 1) * TH)
                            nc.tensor.matmul(
                                psA[:, tsl],
                                lhsT=wkt[0:64, p, st * P:(st + 1) * P],
                                rhs=wqt[0:64, p, tsl],
                                start=True, stop=True, skip_group_check=True)
                            nc.tensor.matmul(
                                psB[:, tsl],
                                lhsT=wkt[64:128, p, st * P:(st + 1) * P],
                                rhs=wqt[64:128, p, tsl],
                                start=True, stop=True, skip_group_check=True)
                        eA = expp.tile([P, T], FP8, tag="exp",
                                       name=f"exA{m}_{p}_{st}")
                        nc.scalar.activation(
                            out=eA[:], in_=psA[:],
                            func=mybir.ActivationFunctionType.Exp,
                            scale=1.0 / (QK_SCALE * QK_SCALE),
                            accum_out=den[:, 0, st:st + 1])
                        eB = expp.tile([P, T], FP8, tag="exp",
                                       name=f"exB{m}_{p}_{st}")
                        nc.scalar.activation(
                            out=eB[:], in_=psB[:],
                            func=mybir.ActivationFunctionType.Exp,
                            scale=1.0 / (QK_SCALE * QK_SCALE),
                            accum_out=den[:, 1, st:st + 1])
                        exA.append(eA)
                        exB.append(eB)
                        if st < 7:
                            maybe_stuff(1)
                    if prev is not None:
                        partial_block(*prev)
                        maybe_stuff(3)
                    prev = (p, exA, exB, den)
                partial_block(*prev)
                if final_stuff:
                    maybe_stuff(3)

            # ---------------- QKV1 projections (PE dense from the start)
            wqt1 = p1.tile([P, 4, T], FP8, tag="wqt", bufs=2, name="wqt1")
            wkt1 = p1.tile([P, 4, T], FP8, tag="wkt", bufs=2, name="wkt1")
            for p in range(4):
                qk_pair(wq_sb, wqt1, p, yT_sb)
                qk_pair(wk_sb, wkt1, p, yT_sb)

            wva1 = p1.tile([P, NT, 512], FP8, tag="wva", bufs=2, name="wva1")
            wva2 = p1.tile([P, NT, 512], FP8, tag="wva", bufs=2, name="wva2")
            h1_sb = p1.tile([P, NF, TH], BF16, tag="h1")

            wq2_sb = p1.tile([P, ND, 512], FP8, tag="wq", name="wq2_sb")
            wk2_sb = p1.tile([P, ND, 512], FP8, tag="wk", name="wk2_sb")
            wv2_sb = p1.tile([P, ND, 512], FP8, tag="wv", name="wv2_sb")
            wqt2 = p1.tile([P, 4, T], FP8, tag="wqt", bufs=2, name="wqt2")
            wkt2 = p1.tile([P, 4, T], FP8, tag="wkt", bufs=2, name="wkt2")

            def load_w2():
                nc.sync.dma_start(wq2_sb[:], wq2[:])
                nc.sync.dma_start(wk2_sb[:], wk2[:])

            def load_wv2():
                nc.sync.dma_start(wv2_sb[:], wv2[:])

            def qk2_units(p):
                return [lambda th=th: qk_half(wq2_sb, wqt2, p, th, yT_sb)
                        for th in range(2)] + \
                       [lambda th=th: qk_half(wk2_sb, wkt2, p, th, xT_sb)
                        for th in range(2)]

            # heads1 stuffing: v1 / v2 / qk2 p0-p3 / h1 (tail spills to heads2)
            for st in range(NT):
                stuff_q.append(lambda st=st: v_unit(wva1, st, yT_sb))
            stuff_q.append(load_w2)
            stuff_q.append(load_wv2)
            stuff_q.extend(qk2_units(0))
            for st in range(NT):
                stuff_q.append(lambda st=st: v_unit(wva2, st, xT_sb))
            stuff_q.extend(qk2_units(1))
            for fc in range(8):
                stuff_q.append(lambda fc=fc: h1_unit(h1_sb, fc))
            stuff_q.extend(qk2_units(2))
            stuff_q.extend(qk2_units(3))
            for fc in range(8, 16):
                stuff_q.append(lambda fc=fc: h1_unit(h1_sb, fc))

            # ---------------- heads1 (scalar-bound; stuffed)
            pt1 = p1.tile([P, 4, T], FP8, tag="pt", bufs=2, name="pt1")
            heads(1, wqt1, wkt1, wva1, pt1, final_stuff=False)

            # ---------------- m1 = pt1 @ Wo1 (full T) -> RS1. The wo1 units
            # go FIRST in the heads2 stuff queue (cheap with DR), so RS1
            # triggers ~15us into heads2; the m1 adds are the LAST stuffed
            # closures, giving the collective the whole phase to land.
            wo1f = p1.tile([P, 4, D], FP8, tag="wof", bufs=2, name="wo1f")
            nc.sync.dma_start(wo1f[:], wo1[:])
            wo2f = p1.tile([P, 4, D], FP8, tag="wof", bufs=2, name="wo2f")
            nc.sync.dma_start(wo2f[:], wo2[:])

            for tt8 in range(NT):
                for dh in range(2):
                    stuff_q.append(lambda tt8=tt8, dh=dh: wo_unit(
                        pt1, wo1f, tt8, dh,
                        rs1_in[tt8 * P:(tt8 + 1) * P, dh * TH:(dh + 1) * TH]))

            def rs1_cc():
                nc.gpsimd.collective_compute(
                    "ReduceScatter", mybir.AluOpType.add, replica_groups=groups,
                    ins=[rs1_in.opt()], outs=[rs1_out.opt()])

            stuff_q.append(rs1_cc)
            for fc in range(16, NF):
                stuff_q.append(lambda fc=fc: h1_unit(h1_sb, fc))

            # ---------------- heads2
            pt2 = p1.tile([P, 4, T], FP8, tag="pt", bufs=2, name="pt2")
            heads(2, wqt2, wkt2, wva2, pt2)
            while stuff_q:
                maybe_stuff(1)

            # ---------------- m2 -> RS2 (hidden under the first ffp sweep).
            # Prefetch the first wot chunks so the sweep starts without
            # waiting behind the wo2 stg DMAs on the sync queue.
            wotc_pre = []
            for fc in range(4):
                wc = p1.tile([P, D], BF16, tag="wotc", bufs=4,
                             name=f"wotc_{fc}")
                nc.sync.dma_start(wc[:], wot[fc])
                wotc_pre.append(wc)
            for k, (tt8, dh) in enumerate((t8, d) for t8 in range(NT)
                                          for d in range(2)):
                wo_unit(pt2, wo2f, tt8, dh,
                        rs2_in[tt8 * P:(tt8 + 1) * P,
                               dh * TH:(dh + 1) * TH],
                        ptag=("st", "pp")[k % 2],
                        cast_eng=("v", "s")[k % 2])
            nc.gpsimd.collective_compute(
                "ReduceScatter", mybir.AluOpType.add, replica_groups=groups,
                ins=[rs2_in.opt()], outs=[rs2_out.opt()])

            def final_chain(tt):
                stats = small.tile([P, 2, 6], FP32, tag="stats",
                                   name=f"stats_{tt}")
                for i in range(2):
                    nc.vector.bn_stats(out=stats[:, i, :],
                                       in_=S[:, tt, i * TH:(i + 1) * TH])
                mv = small.tile([P, 2], FP32, tag="mv", name=f"mv_{tt}")
                nc.vector.bn_aggr(out=mv[:], in_=stats[:])
                std = small.tile([P, 1], FP32, tag="std", name=f"std_{tt}")
                nc.scalar.activation(
                    out=std[:], in_=mv[:, 1:2],
                    func=mybir.ActivationFunctionType.Sqrt,
                    scale=float(D) / float(D - 1))
                msum = small.tile([P, 1], FP32, tag="msum", name=f"msum_{tt}")
                nc.vector.tensor_add(out=msum[:], in0=mv[:, 0:1], in1=std[:])
                nc.vector.tensor_scalar_sub(out=S[:, tt, :], in0=S[:, tt, :],
                                            scalar1=msum[:])
                nc.sync.dma_start(out[tt * P:(tt + 1) * P, :], S[:, tt, :])

            # ---------------- endgame: the ffp sweeps hide RS2; each tt's
            # m1/m2 adds ride after its sweep's S-adds (m1 adds any earlier
            # head-of-line-block the vector queue on RS1 and starve wo2).
            def ffp_sweep(tts):
                acc = [psum.tile([P, T], FP32, tag="sc", name=f"ffa_{tts[0]}_{k}")
                       for k in range(2)]
                for fc in range(NF):
                    if tts[0] == 0 and fc < 4:
                        wot_c = wotc_pre[fc]
                    else:
                        wot_c = p1.tile([P, D], BF16, tag="wotc", bufs=4,
                                        name=f"wotc_{tts[0]}_{fc}")
                        nc.sync.dma_start(wot_c[:], wot[fc])
                    for k, tt in enumerate(tts):
                        for dh in range(2):
                            nc.tensor.matmul(
                                acc[k][:, dh * TH:(dh + 1) * TH],
                                lhsT=h1_sb[:, fc, tt * P:(tt + 1) * P],
                                rhs=wot_c[:, dh * TH:(dh + 1) * TH],
                                start=(fc == 0), stop=(fc == NF - 1),
                                skip_group_check=True)
                for k, tt in enumerate(tts):
                    for dh in range(2):
                        nc.vector.tensor_add(
                            out=S[:, tt, dh * TH:(dh + 1) * TH],
                            in0=acc[k][:, dh * TH:(dh + 1) * TH],
                            in1=S[:, tt, dh * TH:(dh + 1) * TH])

            ffp_sweep((0, 1))
            m_add(rs1_out[0 * P:1 * P, :], 0, "m1")
            m_add(rs1_out[1 * P:2 * P, :], 1, "m1")
            m_add(rs2_out[0 * P:1 * P, :], 0, "m2")
            m_add(rs2_out[1 * P:2 * P, :], 1, "m2")
            final_chain(0)
            final_chain(1)
            # tt2/3's m adds ride under sweep B (RS1/RS2 land before or
            # early in it), leaving only S-adds + chains as the tail
            m_add(rs1_out[2 * P:3 * P, :], 2, "m1")
            m_add(rs1_out[3 * P:4 * P, :], 3, "m1")
            m_add(rs2_out[2 * P:3 * P, :], 2, "m2")
            m_add(rs2_out[3 * P:4 * P, :], 3, "m2")
            ffp_sweep((2, 3))
            final_chain(2)
            final_chain(3)

    if compile:
        nc.compile()
    return nc


# ---------------------------------------------------------------- host side

def pack_inputs(x, y, Wq1, Wk1, Wv1, Wo1, Wq2, Wk2, Wv2, Wo2,
                W_in, b_in, W_out, b_out):
    NH = H // 2
    NPFP8 = mybir.dt.np(FP8)

    def q8(a):
        return np.clip(a, -240.0, 240.0).astype(NPFP8)

    def tr8(a):                # [T, D] -> fp8 [128, ND, T]
        return q8(np.ascontiguousarray(
            a.T.reshape(ND, P, T).transpose(1, 0, 2)))

    def trb(a):                # [T, D] -> bf16 [128, ND, T-slice]
        return np.ascontiguousarray(
            a.T.reshape(ND, P, a.shape[0]).transpose(1, 0, 2)).astype(NPBF16)

    def qk_pack(W, h0):        # [H,D,DK] -> fp8 [128, ND, 512] pair-blocked
        Wh = W[h0:h0 + NH]
        Wp = Wh.reshape(NH // 2, 2, D, DK).transpose(2, 0, 1, 3)
        Wp = Wp.reshape(D, NH * DK)
        return q8(np.ascontiguousarray(
            Wp.reshape(ND, P, NH * DK).transpose(1, 0, 2)))

    def v_pack(W, h0):
        Wh = W[h0:h0 + NH].transpose(1, 0, 2).reshape(D, NH * DV)
        return q8(np.ascontiguousarray(
            Wh.reshape(ND, P, NH * DV).transpose(1, 0, 2)))

    def wo_pack(Wo, h):        # my half rows of Wo -> fp8 [128, 4, D]
        Ws = Wo[NH * DV * h:NH * DV * (h + 1)] * WO_SCALE
        return q8(np.ascontiguousarray(
            Ws.reshape(4, P, D).transpose(1, 0, 2)))

    def wi_pack(W):            # [FF, D] -> bf16 [NF, 128, ND, 128]
        A = W.T.reshape(ND, P, NF, P)
        return np.ascontiguousarray(A.transpose(2, 1, 0, 3)).astype(NPBF16)

    def wot_pack(W):           # [D, FF] -> bf16 [NF, 128, D]
        return np.ascontiguousarray(
            W.T.reshape(NF, P, D)).astype(NPBF16)

    # scores need /sqrt(DK) total; QK_SCALE boost per side is undone by the
    # exp scale (1/QK_SCALE^2).
    sq = np.float32(QK_SCALE / np.sqrt(np.sqrt(np.float32(DK))))

    wi_p = wi_pack(np.asarray(W_in))
    wot_p = wot_pack(np.asarray(W_out))
    bi_p = np.ascontiguousarray(
        np.asarray(b_in).reshape(NF, P).T).astype(np.float32)

    in_maps = []
    for c in range(2 * x.shape[0]):
        b, h = c // 2, c % 2
        h0 = NH * h
        yb, xb = np.asarray(y[b]), np.asarray(x[b])
        in_maps.append(dict(
            yT=tr8(yb),
            xT=tr8(xb),
            yTo=trb(yb)[:, :, h * TH:(h + 1) * TH].copy(),
            ynb=(yb[h * TH:(h + 1) * TH] +
                 np.asarray(b_out)[None, :]).astype(np.float32),
            wq1=qk_pack(Wq1 * sq, h0), wk1=qk_pack(Wk1 * sq, h0),
            wv1=v_pack(Wv1 * WV_SCALE, h0), wo1=wo_pack(np.asarray(Wo1), h),
            wq2=qk_pack(Wq2 * sq, h0), wk2=qk_pack(Wk2 * sq, h0),
            wv2=v_pack(Wv2 * WV_SCALE, h0), wo2=wo_pack(np.asarray(Wo2), h),
            wi=wi_p, wot=wot_p, bi=bi_p,
        ))
    return in_maps


_PROG_CACHE = {}


def kernel(**inputs) -> np.ndarray:
    inputs = {k: np.asarray(v, np.float32) for k, v in inputs.items()}
    if "full" not in _PROG_CACHE:
        _PROG_CACHE["full"] = build_program()
    nc = _PROG_CACHE["full"]
    in_maps = pack_inputs(**inputs)
    res = run_bass_kernel_spmd(nc, in_maps, core_ids=list(range(8)))
    out = np.empty((B, T, D), np.float32)
    for c in range(8):
        b, h = c // 2, c % 2
        out[b, h * TH:(h + 1) * TH] = res.results[c]["out"]
    return out
